# revision 20
# baseline (speedup 1.0000x reference)
"""Expert-parallel top-2 MoE kernel for 8 Trainium2 NeuronCores.

Strategy (expert-parallel, sparse dispatch, per the sharding hint):
  - Router sharded over cores: core c computes fp32 logits for its 512-token
    shard on the TensorEngine (Wg stationary) in [E, 512] layout, AllGathers
    the contiguous [E,512] blocks, and transposes to token-partition layout
    on the TensorEngine; softmax/top-2 on-device (top-2 indicator built with
    5 full-width vector ops via masked second-max).
  - Core c owns expert c. Slot positions come from matmul-based exclusive
    cumsums (single full-width matmuls). Dispatch builds the slot->token map
    with ONE multi-column indirect-DMA scatter of token ids into a [C] DRAM
    vector (tokens not routed here get an out-of-bounds slot id and are
    skipped via bounds_check), loads it back contiguously, transposes it to
    slot-tile layout on the TensorEngine, then indirect-gathers the C routed
    token rows (one op per FFN group) and transposes them into xgT.
  - Two-layer FFN in bf16 over the routed tokens (capacity padded to a
    multiple of 384) in 3 groups of 384 slots; each group's unscaled outputs
    are AllGathered right after they finish, and the combine (one batched
    indirect gather + gate-weighted fp32 accumulation) runs while the next
    group's FFN computes. Only the last chunk's AllGather is exposed.
  - Each core returns its own 512-token shard; host concatenates.

Numerics: router fp32 (top-2 selection fidelity), FFN bf16 with fp32
accumulation in PSUM, combine in fp32.
"""

import os
import sys

import numpy as np

for _p in ("/opt/trn_rl_repo",):
    if _p not in sys.path:
        sys.path.append(_p)

import ml_dtypes

import concourse.bass as bass
import concourse.mybir as mybir
import concourse.tile as tile
from concourse import bacc
from concourse.bass import IndirectOffsetOnAxis
from concourse.masks import make_identity

# Problem shapes (fixed per spec)
B, S, D, E = 2, 2048, 1024, 8
T = B * S          # 4096 tokens
F = 4 * D          # 4096 ffn dim
P = 128            # partitions
NT = T // P        # 32 token tiles
KD = D // P        # 8 contraction tiles over D
NF = F // P        # 32 f tiles
TOK_PER_CORE = T // E   # 512
OWN_TILES = TOK_PER_CORE // P  # 4
N_CORES = E
TG = 3                       # slot tiles per FFN group
CHK = TG * P                 # 384: slot chunk for the chunked AllGather
BIGP = 100000.0              # OOB slot id for tokens not routed here

f32 = mybir.dt.float32
bf16 = mybir.dt.bfloat16
i32 = mybir.dt.int32
u32 = mybir.dt.uint32

_cache = {}


def build_module(C: int, debug_out: bool = False):
    """Build the SPMD Bass module for capacity C (multiple of 384)."""
    assert C % CHK == 0
    ST = C // P  # slot tiles per expert
    NG = ST // TG

    nc = bacc.Bacc("TRN2", target_bir_lowering=False, debug=False,
                   num_devices=N_CORES)

    # ---- I/O ----
    xTs = nc.dram_tensor("xTs", [D, TOK_PER_CORE], f32,
                         kind="ExternalInput").ap()
    xbf = nc.dram_tensor("xbf", [T, D], bf16, kind="ExternalInput").ap()
    w1d = nc.dram_tensor("w1d", [D, F], bf16, kind="ExternalInput").ap()
    w2d = nc.dram_tensor("w2d", [F, D], bf16, kind="ExternalInput").ap()
    wgd = nc.dram_tensor("wgd", [D, E], f32, kind="ExternalInput").ap()
    bgb = nc.dram_tensor("bgb", [P, NT * E], f32, kind="ExternalInput").ap()
    b1pm = nc.dram_tensor("b1pm", [P, NF], f32, kind="ExternalInput").ap()
    b2r = nc.dram_tensor("b2r", [1, D], f32, kind="ExternalInput").ap()
    sel256 = nc.dram_tensor("sel256", [P, NT * E], f32,
                            kind="ExternalInput").ap()
    l128d = nc.dram_tensor("l128d", [P, P], f32, kind="ExternalInput").ap()
    ownmd = nc.dram_tensor("ownmd", [P, OWN_TILES * NT], f32,
                           kind="ExternalInput").ap()
    out = nc.dram_tensor("out", [TOK_PER_CORE, D], f32,
                         kind="ExternalOutput").ap()
    dbg = None
    if debug_out:
        dbg = {
            "dbg_l": nc.dram_tensor("dbg_l", [P, NT * E], f32,
                                    kind="ExternalOutput").ap(),
            "dbg_pos": nc.dram_tensor("dbg_pos", [P, NT * E], f32,
                                      kind="ExternalOutput").ap(),
            "dbg_ind": nc.dram_tensor("dbg_ind", [P, NT * E], f32,
                                      kind="ExternalOutput").ap(),
            "dbg_posm": nc.dram_tensor("dbg_posm", [P, NT], f32,
                                       kind="ExternalOutput").ap(),
            "dbg_idx": nc.dram_tensor("dbg_idx", [P, ST], i32,
                                      kind="ExternalOutput").ap(),
            "dbg_xgT": nc.dram_tensor("dbg_xgT", [P, C], bf16,
                                      kind="ExternalOutput").ap(),
            "dbg_red": nc.dram_tensor(
                "dbg_red", [P, OWN_TILES * 4 * NG], f32,
                kind="ExternalOutput").ap(),
            "dbg_y": [nc.dram_tensor(f"dbg_y{g}", [N_CORES * CHK, D],
                                     bf16, kind="ExternalOutput").ap()
                      for g in range(NG)],
        }

    with tile.TileContext(nc) as tc:
        _emit(tc, C, ST, xTs, xbf, w1d, w2d, wgd, bgb, b1pm, b2r, sel256,
              l128d, ownmd, out, dbg)

    nc.compile()
    return nc


def _emit(tc, C, ST, xTs, xbf, w1d, w2d, wgd, bgb, b1pm, b2r, sel256,
          l128d, ownmd, out, dbg=None):
    nc = tc.nc
    NE = NT * E  # 256
    NG = ST // TG
    NSEL = 2 * NG

    # ---------------- persistent pools ----------------
    persist = tc.alloc_tile_pool(name="persist", bufs=1)
    dram = tc.alloc_tile_pool(name="dram", bufs=1, space="DRAM")

    # tiny warmup AllGather: absorbs first-collective setup cost and aligns
    # the cores before the logits AllGather on the critical path
    wup_in = dram.tile([E, 4], f32, name="wup_in")
    wup_out = dram.tile([N_CORES * E, 4], f32, addr_space="Shared",
                        name="wup_out")
    nc.gpsimd.collective_compute(
        "AllGather", mybir.AluOpType.bypass,
        replica_groups=[list(range(N_CORES))],
        ins=[wup_in[:].opt()], outs=[wup_out[:].opt()],
    )

    # router inputs first (split across DMA queues so the router starts fast)
    wg_sb = persist.tile([P, KD, E], f32, name="wg_sb")
    nc.sync.dma_start(wg_sb[:], wgd.rearrange("(k p) e -> p k e", p=P))
    xs = persist.tile([P, KD, TOK_PER_CORE], f32, name="xs")
    xsv = xTs.rearrange("(k p) t -> p k t", p=P)
    for k in range(KD):
        nc.sync.dma_start(xs[:, k, :], xsv[:, k, :])
    bg_sb = persist.tile([P, NE], f32, name="bg_sb")
    nc.sync.dma_start(bg_sb[:], bgb[:])
    sel_sb = persist.tile([P, NE], f32, name="sel_sb")
    nc.sync.dma_start(sel_sb[:], sel256[:])
    l128_sb = persist.tile([P, P], f32, name="l128_sb")
    nc.sync.dma_start(l128_sb[:], l128d[:])
    ownm_sb = persist.tile([P, OWN_TILES * NT], f32, name="ownm_sb")
    nc.sync.dma_start(ownm_sb[:], ownmd[:])
    b1_sb = persist.tile([P, NF], f32, name="b1_sb")
    nc.sync.dma_start(b1_sb[:], b1pm[:])
    b2_sb = persist.tile([1, D], f32, name="b2_sb")
    nc.sync.dma_start(b2_sb[:], b2r[:])
    ident = persist.tile([P, P], bf16, name="ident")
    make_identity(nc, ident[:])
    identf = persist.tile([P, P], f32, name="identf")
    make_identity(nc, identf[:])
    ones_col = persist.tile([P, 1], f32, name="ones_col")
    nc.vector.memset(ones_col[:], 1.0)
    ones_row = persist.tile([1, P], f32, name="ones_row")
    nc.vector.memset(ones_row[:], 1.0)
    # slot iota (u16: 2x DVE rate) and the [p, tile] pair for idx matmuls
    iotaC = persist.tile([P, C], mybir.dt.uint16, name="iotaC")
    iotaC_i = persist.tile([P, C], i32, name="iotaC_i")
    nc.gpsimd.iota(iotaC_i[:], pattern=[[1, C]], base=0, channel_multiplier=0)
    nc.vector.tensor_copy(iotaC[:], iotaC_i[:])
    pv2 = persist.tile([P, NT, 2], bf16, name="pv2")
    pv2_i = persist.tile([P, NT, 2], i32, name="pv2_i")
    nc.gpsimd.iota(pv2_i[:, :, 0], pattern=[[0, NT]], base=0,
                   channel_multiplier=1)
    nc.gpsimd.iota(pv2_i[:, :, 1], pattern=[[1, NT]], base=0,
                   channel_multiplier=0)
    nc.vector.tensor_copy(pv2[:], pv2_i[:])

    # router / dispatch state kept for the combine phase
    exp_all = persist.tile([P, NE], f32, name="exp_all")    # exp(logits)
    m8_all = persist.tile([P, NE], f32, name="m8_all")      # per-tile top8
    r_all = persist.tile([P, NT], f32, name="r_all")        # 1/sum(exp)
    pos_all = persist.tile([P, NE], f32, name="pos_all")    # excl cumsum
    ind_all = persist.tile([P, NE], f32, name="ind_all")    # top2 indicator
    ei_all = persist.tile([P, NE], u32, name="ei_all")      # top8 indices
    # selection data, plane index k = 2*g + q (group-major for batched gather)
    red_sb = persist.tile([P, OWN_TILES, 2 * NSEL], f32, name="red_sb")
    redi_sb = persist.tile([P, OWN_TILES, NSEL], i32, name="redi_sb")
    ot = [persist.tile([P, D], f32, name=f"ot{j}") for j in range(OWN_TILES)]

    xgT = [persist.tile([P, C], bf16, name=f"xgT{d}") for d in range(KD)]

    l_dram = dram.tile([E, TOK_PER_CORE], f32, name="l_dram")
    lg_dram = dram.tile([N_CORES * E, TOK_PER_CORE], f32, addr_space="Shared",
                        name="lg_dram")
    y_dram = [dram.tile([CHK, D], bf16, name=f"y_dram{g}") for g in range(NG)]
    y_all = [dram.tile([N_CORES * CHK, D], bf16, addr_space="Shared",
                       name=f"y_all{g}") for g in range(NG)]

    # ---------------- router (sharded + AllGather) ----------------
    with tc.tile_pool(name="router_sb", bufs=1, named_scope="router") as rpool, \
         tc.tile_pool(name="router_ps", bufs=1, space="PSUM") as rps:
        lT = rps.tile([E, TOK_PER_CORE], f32, name="lT")
        for k in range(KD):
            nc.tensor.matmul(lT[:], lhsT=wg_sb[:, k, :], rhs=xs[:, k, :],
                             start=(k == 0), stop=(k == KD - 1))
        lt_sb = rpool.tile([E, TOK_PER_CORE], f32, name="lt_sb")
        nc.vector.tensor_copy(lt_sb[:], lT[:])
        nc.sync.dma_start(l_dram[:], lt_sb[:])
        nc.gpsimd.collective_compute(
            "AllGather", mybir.AluOpType.bypass,
            replica_groups=[list(range(N_CORES))],
            ins=[l_dram[:].opt()], outs=[lg_dram[:].opt()],
        )
        # stream gathered logits per core (sources at partition 0), then
        # transpose each 128-token tile into one PSUM tile; single copy out
        l_all = rpool.tile([P, NE], f32, name="l_all")
        QT = TOK_PER_CORE // P  # 4 token tiles per core shard
        pt_all = rps.tile([P, N_CORES, QT, E], f32, name="pt_all")
        for c in range(N_CORES):
            lgc = rpool.tile([E, TOK_PER_CORE], f32, tag="lgc", bufs=2,
                             name="lgc")
            nc.sync.dma_start(lgc[:], lg_dram[c * E:(c + 1) * E, :])
            for q in range(QT):
                nc.tensor.transpose(
                    pt_all[:, c, q, :], lgc[:, q * P:(q + 1) * P],
                    identf[:E, :E])
        nc.vector.tensor_copy(l_all[:], pt_all[:])
        nc.vector.tensor_add(l_all[:], l_all[:], bg_sb[:])
        if dbg is not None:
            nc.sync.dma_start(dbg["dbg_l"][:], l_all[:])
        nc.scalar.activation(exp_all[:], l_all[:],
                             mybir.ActivationFunctionType.Exp)
        # top-2 indicator via masked second-max (5 full-width ops)
        exp3 = exp_all[:].rearrange("p (t e) -> p t e", e=E)
        mx = rpool.tile([P, NT], f32, name="mx")
        nc.vector.reduce_max(mx[:], exp3, axis=mybir.AxisListType.X)
        eqm = rpool.tile([P, NE], f32, name="eqm")
        nc.vector.tensor_tensor(
            out=eqm[:].rearrange("p (t e) -> p t e", e=E), in0=exp3,
            in1=mx[:, :, None].to_broadcast([P, NT, E]),
            op=mybir.AluOpType.is_ge)
        nm = rpool.tile([P, NE], f32, name="nm")
        nc.vector.scalar_tensor_tensor(
            out=nm[:], in0=eqm[:], scalar=-BIGP, in1=exp_all[:],
            op0=mybir.AluOpType.mult, op1=mybir.AluOpType.add)
        m2 = rpool.tile([P, NT], f32, name="m2")
        nc.vector.reduce_max(m2[:], nm[:].rearrange("p (t e) -> p t e", e=E),
                             axis=mybir.AxisListType.X)
        nc.vector.tensor_tensor(
            out=ind_all[:].rearrange("p (t e) -> p t e", e=E), in0=exp3,
            in1=m2[:, :, None].to_broadcast([P, NT, E]),
            op=mybir.AluOpType.is_ge)

    # ---------------- dispatch: cumsum positions + id scatter ----------------
    with tc.tile_pool(name="disp_sb", bufs=1, named_scope="dispatch") as dpool, \
         tc.tile_pool(name="disp_ps", bufs=1, space="PSUM") as dps:
        # per-(tile,expert) totals in one matmul
        ptot = dps.tile([1, NE], f32, name="ptot")
        nc.tensor.matmul(ptot[:], lhsT=ones_col[:], rhs=ind_all[:],
                         start=True, stop=True)
        tot_flat = dpool.tile([1, NE], f32, name="tot_flat")
        nc.vector.tensor_copy(tot_flat[:], ptot[:])
        # reshape [1, NT*E] -> [NT, E] via sbuf-to-sbuf DMA
        tot32 = dpool.tile([NT, E], f32, name="tot32")
        nc.sync.dma_start(tot32[:], tot_flat[:])
        # exclusive cumsum over tiles: strict-lower matmul
        pofs = dps.tile([NT, E], f32, name="pofs")
        nc.tensor.matmul(pofs[:], lhsT=l128_sb[:NT, :NT], rhs=tot32[:],
                         start=True, stop=True)
        ofs32 = dpool.tile([NT, E], f32, name="ofs32")
        nc.vector.tensor_copy(ofs32[:], pofs[:])
        ofs_flat = dpool.tile([1, NE], f32, name="ofs_flat")
        nc.sync.dma_start(ofs_flat[:], ofs32[:])
        # positions: local excl cumsum + broadcast tile offset, one pass
        ppos = dps.tile([P, NE], f32, name="ppos")
        nc.tensor.matmul(ppos[:], lhsT=l128_sb[:], rhs=ind_all[:],
                         start=True, stop=False)
        nc.tensor.matmul(ppos[:], lhsT=ones_row[:], rhs=ofs_flat[:],
                         start=False, stop=True)
        nc.vector.tensor_copy(pos_all[:], ppos[:])

        # my expert's masked positions: ind ? pos : BIGP (skipped as OOB)
        tmp = dpool.tile([P, NE], f32, name="tmp")
        nc.vector.tensor_scalar_add(tmp[:], pos_all[:], -BIGP)
        nc.vector.tensor_mul(tmp[:], tmp[:], ind_all[:])
        nc.vector.tensor_mul(tmp[:], tmp[:], sel_sb[:])
        pos_m = dpool.tile([P, NT], f32, name="pos_m")
        nc.vector.reduce_sum(pos_m[:], tmp[:].rearrange(
            "p (t e) -> p t e", e=E), axis=mybir.AxisListType.X)
        nc.vector.tensor_scalar_add(pos_m[:], pos_m[:], BIGP)
        if dbg is not None:
            nc.sync.dma_start(dbg["dbg_pos"][:], pos_all[:])
            nc.sync.dma_start(dbg["dbg_ind"][:], ind_all[:])
            nc.sync.dma_start(dbg["dbg_posm"][:], pos_m[:])

        # slot->token map via one-hot matmuls, built group by group so the
        # first FFN group's gather can start as early as possible:
        # Pt[p, s] = (pos_m[p,t]==s); acc[0,s] = partition, acc[1,s] = tile
        idx2_sb = dpool.tile([2, C], f32, name="idx2_sb")
        for g in range(NG):
            acc = dps.tile([2, CHK], f32, tag=f"accx{g}", name=f"accx{g}")
            for tt in range(NT):
                Pt = dpool.tile([P, CHK], bf16, tag="Pt", bufs=4, name="Pt")
                nc.vector.tensor_scalar(
                    Pt[:], iotaC[:, g * CHK:(g + 1) * CHK],
                    pos_m[:, tt:tt + 1], None,
                    op0=mybir.AluOpType.is_equal)
                nc.tensor.matmul(acc[:], lhsT=pv2[:, tt, :], rhs=Pt[:],
                                 start=(tt == 0), stop=(tt == NT - 1))
            nc.vector.tensor_copy(idx2_sb[:, g * CHK:(g + 1) * CHK], acc[:])

    # ---- finish idx per group (transpose + fold tile id) + gathers ----
    # xg tiles must outlive this scope (group>0 transposes run inside the
    # FFN section), so they live in a dedicated pool released after FFN.
    xga = [persist.tile([P, D], bf16, name=f"xga{s}") for s in range(ST)]
    with tc.tile_pool(name="gat_sb", bufs=2) as gpool, \
         tc.tile_pool(name="gat_ps", bufs=2, space="PSUM") as gps:
        idx_i = gpool.tile([P, ST], i32, bufs=1, name="idx_i")
        for g in range(NG):
            gsl = slice(g * TG, (g + 1) * TG)
            pti = gps.tile([P, TG, 2], f32, tag="pti", bufs=2, name="pti")
            for t in range(TG):
                s = g * TG + t
                nc.tensor.transpose(pti[:, t, :],
                                    idx2_sb[:, s * P:(s + 1) * P],
                                    identf[:2, :2])
            pti_sb = gpool.tile([P, TG, 2], f32, tag="pti_sb", bufs=2,
                                name="pti_sb")
            nc.vector.tensor_copy(pti_sb[:], pti[:])
            idx_f = gpool.tile([P, TG], f32, tag="idx_f", bufs=2,
                               name="idx_f")
            nc.vector.scalar_tensor_tensor(
                out=idx_f[:], in0=pti_sb[:, :, 1], scalar=float(P),
                in1=pti_sb[:, :, 0], op0=mybir.AluOpType.mult,
                op1=mybir.AluOpType.add)
            nc.vector.tensor_copy(idx_i[:, gsl], idx_f[:])
            for t in range(TG):
                s = g * TG + t
                nc.gpsimd.indirect_dma_start(
                    out=xga[s][:], out_offset=None, in_=xbf[:],
                    in_offset=IndirectOffsetOnAxis(ap=idx_i[:, s:s + 1],
                                                   axis=0),
                )
        if dbg is not None:
            nc.sync.dma_start(dbg["dbg_idx"][:], idx_i[:])
        # group 0 transposes up front (groups 1+ run inside the FFN section)
        for t in range(TG):
            for d in range(KD):
                pt = gps.tile([P, P], bf16, tag="pt", name="pt")
                nc.tensor.transpose(pt[:], xga[t][:, d * P:(d + 1) * P],
                                    ident[:])
                nc.vector.tensor_copy(xgT[d][:, t * P:(t + 1) * P], pt[:])

    with tc.tile_pool(name="sel_sb", bufs=1, named_scope="select") as spool:
        # off the critical path: top-8 values/indices + softmax denom + the
        # combine selection stack (vector work that overlaps the scatter)
        for tt in range(NT):
            sl = slice(tt * E, (tt + 1) * E)
            nc.vector.max(out=m8_all[:, sl], in_=exp_all[:, sl])
            nc.vector.max_index(out=ei_all[:, sl], in_max=m8_all[:, sl],
                                in_values=exp_all[:, sl])
        s_all = spool.tile([P, NT], f32, name="s_all")
        nc.vector.reduce_sum(s_all[:], exp_all[:].rearrange(
            "p (t e) -> p t e", e=E), axis=mybir.AxisListType.X)
        nc.vector.reciprocal(r_all[:], s_all[:])

        # selection stack: NSEL offset planes + NSEL weight planes [P, NT],
        # plane k = 2*g + q (group-major so per-group gathers batch)
        e1f = spool.tile([P, NT], f32, name="e1f")
        e2f = spool.tile([P, NT], f32, name="e2f")
        ei3 = ei_all[:].rearrange("p (t e) -> p t e", e=E)
        nc.vector.tensor_copy(e1f[:], ei3[:, :, 0])
        nc.vector.tensor_copy(e2f[:], ei3[:, :, 1])
        ioz = spool.tile([P, NE], i32, name="ioz")
        nc.gpsimd.iota(ioz[:].rearrange("p (t e) -> p t e", e=E),
                       pattern=[[0, NT], [1, E]], base=0, channel_multiplier=0)
        iof = spool.tile([P, NE], f32, name="iof")
        nc.vector.tensor_copy(iof[:], ioz[:])
        m83 = m8_all[:].rearrange("p (t e) -> p t e", e=E)
        Ssel = spool.tile([P, 2 * NSEL, NT], f32, name="Ssel")
        for q, ef in ((0, e1f), (1, e2f)):
            oh = spool.tile([P, NE], f32, tag=f"oh{q}", name=f"oh{q}")
            nc.vector.tensor_tensor(
                out=oh[:].rearrange("p (t e) -> p t e", e=E),
                in0=iof[:].rearrange("p (t e) -> p t e", e=E),
                in1=ef[:, :, None].to_broadcast([P, NT, E]),
                op=mybir.AluOpType.is_equal)
            nc.vector.tensor_mul(oh[:], oh[:], pos_all[:])
            slot = spool.tile([P, NT], f32, tag=f"slot{q}", name=f"slot{q}")
            nc.vector.reduce_sum(slot[:], oh[:].rearrange(
                "p (t e) -> p t e", e=E), axis=mybir.AxisListType.X)
            gch = spool.tile([P, NT], f32, tag=f"gch{q}", name=f"gch{q}")
            nc.vector.tensor_scalar(gch[:], slot[:], float(CHK), None,
                                    op0=mybir.AluOpType.is_ge)
            for gg in range(2, NG):
                t2 = spool.tile([P, NT], f32, tag="t2", name="t2")
                nc.vector.tensor_scalar(t2[:], slot[:], float(CHK * gg), None,
                                        op0=mybir.AluOpType.is_ge)
                nc.vector.tensor_add(gch[:], gch[:], t2[:])
            base = spool.tile([P, NT], f32, tag=f"base{q}", name=f"base{q}")
            nc.vector.scalar_tensor_tensor(
                out=base[:], in0=ef[:], scalar=float(CHK), in1=slot[:],
                op0=mybir.AluOpType.mult, op1=mybir.AluOpType.add)
            gv = spool.tile([P, NT], f32, tag=f"gv{q}", name=f"gv{q}")
            nc.vector.tensor_tensor(out=gv[:], in0=m83[:, :, q], in1=r_all[:],
                                    op=mybir.AluOpType.mult)
            for gg in range(NG):
                k = 2 * gg + q
                eq = spool.tile([P, NT], f32, tag="eq", name="eq")
                nc.vector.tensor_scalar(eq[:], gch[:], float(gg), None,
                                        op0=mybir.AluOpType.is_equal)
                nc.vector.tensor_scalar_add(Ssel[:, k, :], base[:],
                                            float(-CHK * gg))
                nc.vector.tensor_mul(Ssel[:, k, :], Ssel[:, k, :], eq[:])
                nc.vector.tensor_mul(Ssel[:, NSEL + k, :], eq[:], gv[:])
        for j in range(OWN_TILES):
            own = ownm_sb[:, j * NT:(j + 1) * NT]
            tmpS = spool.tile([P, 2 * NSEL, NT], f32, tag="tmpS", bufs=2,
                              name="tmpS")
            nc.vector.tensor_tensor(
                out=tmpS[:], in0=Ssel[:],
                in1=own[:, None, :].to_broadcast([P, 2 * NSEL, NT]),
                op=mybir.AluOpType.mult)
            nc.vector.reduce_sum(red_sb[:, j, :], tmpS[:],
                                 axis=mybir.AxisListType.X)
            nc.vector.tensor_copy(redi_sb[:, j, :], red_sb[:, j, :NSEL])
        if dbg is not None:
            nc.sync.dma_start(
                dbg["dbg_red"][:].rearrange("p (j k) -> p j k", k=2 * NSEL),
                red_sb[:])


    # -------- FFN (bf16) + chunked y AllGather + overlapped combine --------
    with tc.tile_pool(name="ffn_sb", bufs=1, named_scope="ffn") as fpool, \
         tc.tile_pool(name="ffn_ps", bufs=1, space="PSUM") as fps, \
         tc.tile_pool(name="comb_sb", bufs=2, named_scope="combine") as cpool:
        w1v = w1d.rearrange("(k p) f -> p k f", p=P)
        for g in range(NG):
            t0 = g * TG
            py = [[fps.tile([P, 512], f32, tag=f"py_{t}_{n}",
                            name=f"py_{t}_{n}")
                   for n in range(2)] for t in range(TG)]
            for f in range(NF):
                w1f = fpool.tile([P, KD, P], bf16, tag="w1f", bufs=6,
                                 name="w1f")
                nc.sync.dma_start(w1f[:], w1v[:, :, f * P:(f + 1) * P])
                ph = fps.tile([P, CHK], f32, tag="ph", bufs=1, name="ph")
                for k in range(KD):
                    nc.tensor.matmul(
                        ph[:], lhsT=w1f[:, k, :],
                        rhs=xgT[k][:, t0 * P:t0 * P + CHK],
                        start=(k == 0), stop=(k == KD - 1))
                hbuf = fpool.tile([P, CHK], bf16, tag="hbuf", bufs=3,
                                  name="hbuf")
                nc.scalar.activation(hbuf[:], ph[:],
                                     mybir.ActivationFunctionType.Relu,
                                     bias=b1_sb[:, f:f + 1], scale=1.0)
                w2f = fpool.tile([P, D], bf16, tag="w2f", bufs=6, name="w2f")
                nc.sync.dma_start(w2f[:], w2d[f * P:(f + 1) * P, :])
                for t in range(TG):
                    for n in range(2):
                        nc.tensor.matmul(
                            py[t][n][:],
                            lhsT=hbuf[:, t * P:(t + 1) * P],
                            rhs=w2f[:, n * 512:(n + 1) * 512],
                            start=(f == 0), stop=False)
                # mid-group: transpose the NEXT group's gathered tokens into
                # xgT using the PSUM bank freed by ph bufs=1
                if f == 20 and g + 1 < NG:
                    for t in range(TG):
                        s = (g + 1) * TG + t
                        for d in range(KD):
                            pt = fps.tile([P, P], bf16, tag="ptn", bufs=1,
                                          name="ptn")
                            nc.tensor.transpose(
                                pt[:], xga[s][:, d * P:(d + 1) * P],
                                ident[:])
                            nc.vector.tensor_copy(
                                xgT[d][:, s * P:(s + 1) * P], pt[:])
            # bias b2 via rank-1 matmul, then write out
            for t in range(TG):
                for n in range(2):
                    nc.tensor.matmul(
                        py[t][n][:], lhsT=ones_row[:],
                        rhs=b2_sb[:, n * 512:(n + 1) * 512],
                        start=False, stop=True)
                ysb = fpool.tile([P, D], bf16, tag="ysb", bufs=2, name="ysb")
                nc.vector.tensor_copy(ysb[:, :512], py[t][0][:])
                nc.vector.tensor_copy(ysb[:, 512:], py[t][1][:])
                nc.sync.dma_start(y_dram[g][t * P:(t + 1) * P, :], ysb[:])
            # ship this chunk while the next group computes
            nc.gpsimd.collective_compute(
                "AllGather", mybir.AluOpType.bypass,
                replica_groups=[list(range(N_CORES))],
                ins=[y_dram[g][:].opt()],
                outs=[y_all[g][:].opt()],
            )
            if dbg is not None:
                nc.sync.dma_start(dbg["dbg_y"][g][:], y_all[g][:])
            # combine this chunk: one batched gather of all (j, q) rows,
            # then gate-weighted accumulation while the next group runs
            yt = cpool.tile([P, OWN_TILES * 2, D], bf16, tag=f"yt{g}",
                            bufs=1, name=f"yt{g}")
            for j in range(OWN_TILES):
                for q in range(2):
                    nc.gpsimd.indirect_dma_start(
                        out=yt[:, 2 * j + q, :], out_offset=None,
                        in_=y_all[g][:],
                        in_offset=IndirectOffsetOnAxis(
                            ap=redi_sb[:, j, 2 * g + q:2 * g + q + 1],
                            axis=0))
            for j in range(OWN_TILES):
                for q in range(2):
                    k = 2 * g + q
                    w = red_sb[:, j, NSEL + k:NSEL + k + 1]
                    if g == 0 and q == 0:
                        nc.vector.tensor_scalar(ot[j][:], yt[:, 2 * j + q, :],
                                                w, None,
                                                op0=mybir.AluOpType.mult)
                    else:
                        nc.vector.scalar_tensor_tensor(
                            out=ot[j][:], in0=yt[:, 2 * j + q, :], scalar=w,
                            in1=ot[j][:], op0=mybir.AluOpType.mult,
                            op1=mybir.AluOpType.add)
        for j in range(OWN_TILES):
            nc.sync.dma_start(out[j * P:(j + 1) * P, :], ot[j][:])

    persist.release()
    dram.release()


def _host_prep(x, Wg, bg, W1, b1, W2, b2, C):
    xf = np.ascontiguousarray(x.reshape(T, D).astype(np.float32))
    xT = np.ascontiguousarray(xf.T)
    xbf = xf.astype(ml_dtypes.bfloat16)
    bgb = np.tile(bg.astype(np.float32), NT)[None, :].repeat(P, 0)
    bgb = np.ascontiguousarray(bgb)
    l128 = np.triu(np.ones((P, P), np.float32), 1)  # [t', t] = 1 if t' < t
    in_maps = []
    for c in range(N_CORES):
        sel = np.zeros(E, np.float32)
        sel[c] = 1.0
        sel256 = np.ascontiguousarray(np.tile(sel, NT)[None, :].repeat(P, 0))
        ownm = np.zeros((P, OWN_TILES, NT), np.float32)
        for j in range(OWN_TILES):
            ownm[:, j, OWN_TILES * c + j] = 1.0
        in_maps.append({
            "xTs": np.ascontiguousarray(
                xT[:, c * TOK_PER_CORE:(c + 1) * TOK_PER_CORE]),
            "xbf": xbf,
            "w1d": np.ascontiguousarray(W1[c].astype(ml_dtypes.bfloat16)),
            "w2d": np.ascontiguousarray(W2[c].astype(ml_dtypes.bfloat16)),
            "wgd": np.ascontiguousarray(Wg.astype(np.float32)),
            "bgb": bgb,
            "b1pm": np.ascontiguousarray(
                b1[c].astype(np.float32).reshape(NF, P).T),
            "b2r": np.ascontiguousarray(b2[c].astype(np.float32)[None, :]),
            "sel256": sel256,
            "l128d": l128,
            "ownmd": np.ascontiguousarray(ownm.reshape(P, OWN_TILES * NT)),
        })
    return in_maps


def _capacity(x, Wg, bg):
    xf = x.reshape(T, D).astype(np.float32)
    logits = xf @ Wg.astype(np.float32) + bg.astype(np.float32)
    part = np.partition(logits, E - 2, axis=-1)
    m2 = part[:, E - 2:E - 1]
    counts = (logits >= m2).sum(0)
    return int(np.ceil((counts.max() + 16) / CHK) * CHK)


LAST_RESULT = None


def kernel(x, Wg, bg, W1, b1, W2, b2):
    global LAST_RESULT
    from concourse.bass_utils import run_bass_kernel_spmd

    x = np.asarray(x)
    C = _capacity(x, np.asarray(Wg), np.asarray(bg))
    debug = bool(os.environ.get("BASS_DEBUG_OUT"))
    key = (C, debug)
    if key not in _cache:
        _cache[key] = build_module(C, debug_out=debug)
    nc = _cache[key]
    in_maps = _host_prep(x, np.asarray(Wg), np.asarray(bg), np.asarray(W1),
                         np.asarray(b1), np.asarray(W2), np.asarray(b2), C)
    trace = bool(os.environ.get("BASS_TRACE"))
    if trace:
        _setup_axon_profile_hook()
    res = run_bass_kernel_spmd(nc, in_maps, core_ids=list(range(N_CORES)),
                               trace=trace)
    LAST_RESULT = res
    out = np.empty((T, D), np.float32)
    for c in range(N_CORES):
        out[c * TOK_PER_CORE:(c + 1) * TOK_PER_CORE] = res.results[c]["out"]
    return out.reshape(B, S, D)


def _setup_axon_profile_hook():
    """Provide antenv.axon_hooks (missing in this image) so trace=True works."""
    import types
    try:
        import antenv
        if "antenv.axon_hooks" not in sys.modules:
            hooks = types.ModuleType("antenv.axon_hooks")
            hooks._hook = None
            hooks.set_axon_ntff_profile_hook = \
                lambda h: setattr(hooks, "_hook", h)
            hooks.get_axon_ntff_profile_hook = lambda: hooks._hook
            sys.modules["antenv.axon_hooks"] = hooks
            antenv.axon_hooks = hooks
            from trn_agent_boot.trn_boot import _ntff_profile_via_ctypes
            hooks.set_axon_ntff_profile_hook(
                _ntff_profile_via_ctypes("/opt/axon/libaxon_pjrt.so"))
    except Exception as e:  # profiling is best-effort
        print(f"profile hook setup failed: {e}", file=sys.stderr)


# revision 21
# speedup vs baseline: 1.0150x; 1.0150x over previous
"""Expert-parallel top-2 MoE kernel for 8 Trainium2 NeuronCores.

Strategy (expert-parallel, sparse dispatch, per the sharding hint):
  - Router sharded over cores: core c computes fp32 logits for its 512-token
    shard on the TensorEngine (Wg stationary) in [E, 512] layout, AllGathers
    the contiguous [E,512] blocks, and transposes to token-partition layout
    on the TensorEngine; softmax/top-2 on-device (top-2 indicator built with
    5 full-width vector ops via masked second-max).
  - Core c owns expert c. Slot positions come from matmul-based exclusive
    cumsums (single full-width matmuls). Dispatch builds the slot->token map
    with ONE multi-column indirect-DMA scatter of token ids into a [C] DRAM
    vector (tokens not routed here get an out-of-bounds slot id and are
    skipped via bounds_check), loads it back contiguously, transposes it to
    slot-tile layout on the TensorEngine, then indirect-gathers the C routed
    token rows (one op per FFN group) and transposes them into xgT.
  - Two-layer FFN in bf16 over the routed tokens (capacity padded to a
    multiple of 384) in 3 groups of 384 slots; each group's unscaled outputs
    are AllGathered right after they finish, and the combine (one batched
    indirect gather + gate-weighted fp32 accumulation) runs while the next
    group's FFN computes. Only the last chunk's AllGather is exposed.
  - Each core returns its own 512-token shard; host concatenates.

Numerics: router fp32 (top-2 selection fidelity), FFN bf16 with fp32
accumulation in PSUM, combine in fp32.
"""

import os
import sys

import numpy as np

for _p in ("/opt/trn_rl_repo",):
    if _p not in sys.path:
        sys.path.append(_p)

import ml_dtypes

import concourse.bass as bass
import concourse.mybir as mybir
import concourse.tile as tile
from concourse import bacc
from concourse.bass import IndirectOffsetOnAxis
from concourse.masks import make_identity

# Problem shapes (fixed per spec)
B, S, D, E = 2, 2048, 1024, 8
T = B * S          # 4096 tokens
F = 4 * D          # 4096 ffn dim
P = 128            # partitions
NT = T // P        # 32 token tiles
KD = D // P        # 8 contraction tiles over D
NF = F // P        # 32 f tiles
TOK_PER_CORE = T // E   # 512
OWN_TILES = TOK_PER_CORE // P  # 4
N_CORES = E
TG = 3                       # slot tiles per FFN group
CHK = TG * P                 # 384: slot chunk for the chunked AllGather
BIGP = 100000.0              # OOB slot id for tokens not routed here

f32 = mybir.dt.float32
bf16 = mybir.dt.bfloat16
i32 = mybir.dt.int32
u32 = mybir.dt.uint32

_cache = {}


def build_module(C: int, debug_out: bool = False):
    """Build the SPMD Bass module for capacity C (multiple of 384)."""
    assert C % CHK == 0
    ST = C // P  # slot tiles per expert
    NG = ST // TG

    nc = bacc.Bacc("TRN2", target_bir_lowering=False, debug=False,
                   num_devices=N_CORES)

    # ---- I/O ----
    xTf = nc.dram_tensor("xTf", [D, T], f32, kind="ExternalInput").ap()
    xbf = nc.dram_tensor("xbf", [T, D], bf16, kind="ExternalInput").ap()
    w1d = nc.dram_tensor("w1d", [D, F], bf16, kind="ExternalInput").ap()
    w2d = nc.dram_tensor("w2d", [F, D], bf16, kind="ExternalInput").ap()
    wgd = nc.dram_tensor("wgd", [D, E], f32, kind="ExternalInput").ap()
    bgb = nc.dram_tensor("bgb", [P, NT * E], f32, kind="ExternalInput").ap()
    b1pm = nc.dram_tensor("b1pm", [P, NF], f32, kind="ExternalInput").ap()
    b2r = nc.dram_tensor("b2r", [1, D], f32, kind="ExternalInput").ap()
    sel256 = nc.dram_tensor("sel256", [P, NT * E], f32,
                            kind="ExternalInput").ap()
    l128d = nc.dram_tensor("l128d", [P, P], f32, kind="ExternalInput").ap()
    ownmd = nc.dram_tensor("ownmd", [P, OWN_TILES * NT], f32,
                           kind="ExternalInput").ap()
    out = nc.dram_tensor("out", [TOK_PER_CORE, D], f32,
                         kind="ExternalOutput").ap()
    dbg = None
    if debug_out:
        dbg = {
            "dbg_l": nc.dram_tensor("dbg_l", [P, NT * E], f32,
                                    kind="ExternalOutput").ap(),
            "dbg_pos": nc.dram_tensor("dbg_pos", [P, NT * E], f32,
                                      kind="ExternalOutput").ap(),
            "dbg_ind": nc.dram_tensor("dbg_ind", [P, NT * E], f32,
                                      kind="ExternalOutput").ap(),
            "dbg_posm": nc.dram_tensor("dbg_posm", [P, NT], f32,
                                       kind="ExternalOutput").ap(),
            "dbg_idx": nc.dram_tensor("dbg_idx", [P, ST], i32,
                                      kind="ExternalOutput").ap(),
            "dbg_xgT": nc.dram_tensor("dbg_xgT", [P, C], bf16,
                                      kind="ExternalOutput").ap(),
            "dbg_red": nc.dram_tensor(
                "dbg_red", [P, OWN_TILES * 4 * NG], f32,
                kind="ExternalOutput").ap(),
            "dbg_y": [nc.dram_tensor(f"dbg_y{g}", [N_CORES * CHK, D],
                                     bf16, kind="ExternalOutput").ap()
                      for g in range(NG)],
        }

    with tile.TileContext(nc) as tc:
        _emit(tc, C, ST, xTf, xbf, w1d, w2d, wgd, bgb, b1pm, b2r, sel256,
              l128d, ownmd, out, dbg)

    nc.compile()
    return nc


def _emit(tc, C, ST, xTf, xbf, w1d, w2d, wgd, bgb, b1pm, b2r, sel256,
          l128d, ownmd, out, dbg=None):
    nc = tc.nc
    NE = NT * E  # 256
    NG = ST // TG
    NSEL = 2 * NG

    # ---------------- persistent pools ----------------
    persist = tc.alloc_tile_pool(name="persist", bufs=1)
    dram = tc.alloc_tile_pool(name="dram", bufs=1, space="DRAM")

    # tiny warmup AllGather: absorbs first-collective setup cost and aligns
    # the cores before the logits AllGather on the critical path
    wup_in = dram.tile([E, 4], f32, name="wup_in")
    wup_out = dram.tile([N_CORES * E, 4], f32, addr_space="Shared",
                        name="wup_out")
    nc.gpsimd.collective_compute(
        "AllGather", mybir.AluOpType.bypass,
        replica_groups=[list(range(N_CORES))],
        ins=[wup_in[:].opt()], outs=[wup_out[:].opt()],
    )

    # router inputs first (split across DMA queues so the router starts fast)
    wg_sb = persist.tile([P, KD, E], f32, name="wg_sb")
    nc.sync.dma_start(wg_sb[:], wgd.rearrange("(k p) e -> p k e", p=P))
    bg_sb = persist.tile([P, NE], f32, name="bg_sb")
    nc.sync.dma_start(bg_sb[:], bgb[:])
    sel_sb = persist.tile([P, NE], f32, name="sel_sb")
    nc.sync.dma_start(sel_sb[:], sel256[:])
    l128_sb = persist.tile([P, P], f32, name="l128_sb")
    nc.sync.dma_start(l128_sb[:], l128d[:])
    ownm_sb = persist.tile([P, OWN_TILES * NT], f32, name="ownm_sb")
    nc.sync.dma_start(ownm_sb[:], ownmd[:])
    b1_sb = persist.tile([P, NF], f32, name="b1_sb")
    nc.sync.dma_start(b1_sb[:], b1pm[:])
    b2_sb = persist.tile([1, D], f32, name="b2_sb")
    nc.sync.dma_start(b2_sb[:], b2r[:])
    w1_sb = [persist.tile([P, F], bf16, name=f"w1_sb{k}") for k in range(KD)]
    for k in range(KD):
        nc.sync.dma_start(w1_sb[k][:], w1d[k * P:(k + 1) * P, :])
    ident = persist.tile([P, P], bf16, name="ident")
    make_identity(nc, ident[:])
    identf = persist.tile([P, P], f32, name="identf")
    make_identity(nc, identf[:])
    ones_col = persist.tile([P, 1], f32, name="ones_col")
    nc.vector.memset(ones_col[:], 1.0)
    ones_row = persist.tile([1, P], f32, name="ones_row")
    nc.vector.memset(ones_row[:], 1.0)
    # slot iota (u16: 2x DVE rate) and the [p, tile] pair for idx matmuls
    iotaC = persist.tile([P, C], mybir.dt.uint16, name="iotaC")
    iotaC_i = persist.tile([P, C], i32, name="iotaC_i")
    nc.gpsimd.iota(iotaC_i[:], pattern=[[1, C]], base=0, channel_multiplier=0)
    nc.vector.tensor_copy(iotaC[:], iotaC_i[:])
    pv2 = persist.tile([P, NT, 2], bf16, name="pv2")
    pv2_i = persist.tile([P, NT, 2], i32, name="pv2_i")
    nc.gpsimd.iota(pv2_i[:, :, 0], pattern=[[0, NT]], base=0,
                   channel_multiplier=1)
    nc.gpsimd.iota(pv2_i[:, :, 1], pattern=[[1, NT]], base=0,
                   channel_multiplier=0)
    nc.vector.tensor_copy(pv2[:], pv2_i[:])

    # router / dispatch state kept for the combine phase
    exp_all = persist.tile([P, NE], f32, name="exp_all")    # exp(logits)
    m8_all = persist.tile([P, NE], f32, name="m8_all")      # per-tile top8
    r_all = persist.tile([P, NT], f32, name="r_all")        # 1/sum(exp)
    pos_all = persist.tile([P, NE], f32, name="pos_all")    # excl cumsum
    ind_all = persist.tile([P, NE], f32, name="ind_all")    # top2 indicator
    ei_all = persist.tile([P, NE], u32, name="ei_all")      # top8 indices
    # selection data, plane index k = 2*g + q (group-major for batched gather)
    red_sb = persist.tile([P, OWN_TILES, 2 * NSEL], f32, name="red_sb")
    redi_sb = persist.tile([P, OWN_TILES, NSEL], i32, name="redi_sb")
    ot = [persist.tile([P, D], f32, name=f"ot{j}") for j in range(OWN_TILES)]

    xgT = [persist.tile([P, C], bf16, name=f"xgT{d}") for d in range(KD)]

    y_dram = [dram.tile([CHK, D], bf16, name=f"y_dram{g}") for g in range(NG)]
    y_all = [dram.tile([N_CORES * CHK, D], bf16, addr_space="Shared",
                       name=f"y_all{g}") for g in range(NG)]

    # ------------- router: every core computes ALL logits locally -------------
    # (full xT fp32 is an input; no logits collective on the critical path)
    with tc.tile_pool(name="router_sb", bufs=1, named_scope="router") as rpool, \
         tc.tile_pool(name="router_ps", bufs=1, space="PSUM") as rps:
        l_all = rpool.tile([P, NE], f32, name="l_all")
        xTv = xTf.rearrange("(k p) t -> p k t", p=P)
        NCH = T // TOK_PER_CORE  # 8 chunks of 512 tokens
        QT = TOK_PER_CORE // P   # 4 token tiles per chunk
        for c in range(NCH):
            xsc = rpool.tile([P, KD, TOK_PER_CORE], f32, tag="xsc", bufs=2,
                             name="xsc")
            for k in range(KD):
                nc.sync.dma_start(
                    xsc[:, k, :],
                    xTv[:, k, c * TOK_PER_CORE:(c + 1) * TOK_PER_CORE])
            lT = rps.tile([E, TOK_PER_CORE], f32, tag="lT", bufs=2, name="lT")
            for k in range(KD):
                nc.tensor.matmul(lT[:], lhsT=wg_sb[:, k, :], rhs=xsc[:, k, :],
                                 start=(k == 0), stop=(k == KD - 1))
            lsb = rpool.tile([E, TOK_PER_CORE], f32, tag="lsb", bufs=2,
                             name="lsb")
            nc.vector.tensor_copy(lsb[:], lT[:])
            pt_l = rps.tile([P, QT, E], f32, tag="pt_l", bufs=2, name="pt_l")
            for q in range(QT):
                nc.tensor.transpose(
                    pt_l[:, q, :], lsb[:, q * P:(q + 1) * P], identf[:E, :E])
            nc.vector.tensor_copy(
                l_all[:, c * QT * E:(c + 1) * QT * E], pt_l[:])
        nc.vector.tensor_add(l_all[:], l_all[:], bg_sb[:])
        if dbg is not None:
            nc.sync.dma_start(dbg["dbg_l"][:], l_all[:])
        nc.scalar.activation(exp_all[:], l_all[:],
                             mybir.ActivationFunctionType.Exp)
        # top-2 indicator via masked second-max (5 full-width ops)
        exp3 = exp_all[:].rearrange("p (t e) -> p t e", e=E)
        mx = rpool.tile([P, NT], f32, name="mx")
        nc.vector.reduce_max(mx[:], exp3, axis=mybir.AxisListType.X)
        eqm = rpool.tile([P, NE], f32, name="eqm")
        nc.vector.tensor_tensor(
            out=eqm[:].rearrange("p (t e) -> p t e", e=E), in0=exp3,
            in1=mx[:, :, None].to_broadcast([P, NT, E]),
            op=mybir.AluOpType.is_ge)
        nm = rpool.tile([P, NE], f32, name="nm")
        nc.vector.scalar_tensor_tensor(
            out=nm[:], in0=eqm[:], scalar=-BIGP, in1=exp_all[:],
            op0=mybir.AluOpType.mult, op1=mybir.AluOpType.add)
        m2 = rpool.tile([P, NT], f32, name="m2")
        nc.vector.reduce_max(m2[:], nm[:].rearrange("p (t e) -> p t e", e=E),
                             axis=mybir.AxisListType.X)
        nc.vector.tensor_tensor(
            out=ind_all[:].rearrange("p (t e) -> p t e", e=E), in0=exp3,
            in1=m2[:, :, None].to_broadcast([P, NT, E]),
            op=mybir.AluOpType.is_ge)

    # ---------------- dispatch: cumsum positions + id scatter ----------------
    with tc.tile_pool(name="disp_sb", bufs=1, named_scope="dispatch") as dpool, \
         tc.tile_pool(name="disp_ps", bufs=1, space="PSUM") as dps:
        # per-(tile,expert) totals in one matmul
        ptot = dps.tile([1, NE], f32, name="ptot")
        nc.tensor.matmul(ptot[:], lhsT=ones_col[:], rhs=ind_all[:],
                         start=True, stop=True)
        tot_flat = dpool.tile([1, NE], f32, name="tot_flat")
        nc.vector.tensor_copy(tot_flat[:], ptot[:])
        # reshape [1, NT*E] -> [NT, E] via sbuf-to-sbuf DMA
        tot32 = dpool.tile([NT, E], f32, name="tot32")
        nc.sync.dma_start(tot32[:], tot_flat[:])
        # exclusive cumsum over tiles: strict-lower matmul
        pofs = dps.tile([NT, E], f32, name="pofs")
        nc.tensor.matmul(pofs[:], lhsT=l128_sb[:NT, :NT], rhs=tot32[:],
                         start=True, stop=True)
        ofs32 = dpool.tile([NT, E], f32, name="ofs32")
        nc.vector.tensor_copy(ofs32[:], pofs[:])
        ofs_flat = dpool.tile([1, NE], f32, name="ofs_flat")
        nc.sync.dma_start(ofs_flat[:], ofs32[:])
        # positions: local excl cumsum + broadcast tile offset, one pass
        ppos = dps.tile([P, NE], f32, name="ppos")
        nc.tensor.matmul(ppos[:], lhsT=l128_sb[:], rhs=ind_all[:],
                         start=True, stop=False)
        nc.tensor.matmul(ppos[:], lhsT=ones_row[:], rhs=ofs_flat[:],
                         start=False, stop=True)
        nc.vector.tensor_copy(pos_all[:], ppos[:])

        # my expert's masked positions: ind ? pos : BIGP (skipped as OOB)
        tmp = dpool.tile([P, NE], f32, name="tmp")
        nc.vector.tensor_scalar_add(tmp[:], pos_all[:], -BIGP)
        nc.vector.tensor_mul(tmp[:], tmp[:], ind_all[:])
        nc.vector.tensor_mul(tmp[:], tmp[:], sel_sb[:])
        pos_m = dpool.tile([P, NT], f32, name="pos_m")
        nc.vector.reduce_sum(pos_m[:], tmp[:].rearrange(
            "p (t e) -> p t e", e=E), axis=mybir.AxisListType.X)
        nc.vector.tensor_scalar_add(pos_m[:], pos_m[:], BIGP)
        if dbg is not None:
            nc.sync.dma_start(dbg["dbg_pos"][:], pos_all[:])
            nc.sync.dma_start(dbg["dbg_ind"][:], ind_all[:])
            nc.sync.dma_start(dbg["dbg_posm"][:], pos_m[:])

        # slot->token map via one-hot matmuls, built group by group so the
        # first FFN group's gather can start as early as possible:
        # Pt[p, s] = (pos_m[p,t]==s); acc[0,s] = partition, acc[1,s] = tile
        idx2_sb = dpool.tile([2, C], f32, name="idx2_sb")
        for g in range(NG):
            acc = dps.tile([2, CHK], f32, tag=f"accx{g}", name=f"accx{g}")
            for tt in range(NT):
                Pt = dpool.tile([P, CHK], bf16, tag="Pt", bufs=4, name="Pt")
                nc.vector.tensor_scalar(
                    Pt[:], iotaC[:, g * CHK:(g + 1) * CHK],
                    pos_m[:, tt:tt + 1], None,
                    op0=mybir.AluOpType.is_equal)
                nc.tensor.matmul(acc[:], lhsT=pv2[:, tt, :], rhs=Pt[:],
                                 start=(tt == 0), stop=(tt == NT - 1))
            nc.vector.tensor_copy(idx2_sb[:, g * CHK:(g + 1) * CHK], acc[:])

    # ---- finish idx per group (transpose + fold tile id) + gathers ----
    # xg tiles must outlive this scope (group>0 transposes run inside the
    # FFN section), so they live in a dedicated pool released after FFN.
    xga = [persist.tile([P, D], bf16, name=f"xga{s}") for s in range(ST)]
    with tc.tile_pool(name="gat_sb", bufs=2) as gpool, \
         tc.tile_pool(name="gat_ps", bufs=2, space="PSUM") as gps:
        idx_i = gpool.tile([P, ST], i32, bufs=1, name="idx_i")
        for g in range(NG):
            gsl = slice(g * TG, (g + 1) * TG)
            pti = gps.tile([P, TG, 2], f32, tag="pti", bufs=2, name="pti")
            for t in range(TG):
                s = g * TG + t
                nc.tensor.transpose(pti[:, t, :],
                                    idx2_sb[:, s * P:(s + 1) * P],
                                    identf[:2, :2])
            pti_sb = gpool.tile([P, TG, 2], f32, tag="pti_sb", bufs=2,
                                name="pti_sb")
            nc.vector.tensor_copy(pti_sb[:], pti[:])
            idx_f = gpool.tile([P, TG], f32, tag="idx_f", bufs=2,
                               name="idx_f")
            nc.vector.scalar_tensor_tensor(
                out=idx_f[:], in0=pti_sb[:, :, 1], scalar=float(P),
                in1=pti_sb[:, :, 0], op0=mybir.AluOpType.mult,
                op1=mybir.AluOpType.add)
            nc.vector.tensor_copy(idx_i[:, gsl], idx_f[:])
            for t in range(TG):
                s = g * TG + t
                nc.gpsimd.indirect_dma_start(
                    out=xga[s][:], out_offset=None, in_=xbf[:],
                    in_offset=IndirectOffsetOnAxis(ap=idx_i[:, s:s + 1],
                                                   axis=0),
                )
        if dbg is not None:
            nc.sync.dma_start(dbg["dbg_idx"][:], idx_i[:])
        # group 0 transposes up front (groups 1+ run inside the FFN section)
        for t in range(TG):
            for d in range(KD):
                pt = gps.tile([P, P], bf16, tag="pt", name="pt")
                nc.tensor.transpose(pt[:], xga[t][:, d * P:(d + 1) * P],
                                    ident[:])
                nc.vector.tensor_copy(xgT[d][:, t * P:(t + 1) * P], pt[:])

    with tc.tile_pool(name="sel_sb", bufs=1, named_scope="select") as spool:
        # off the critical path: top-8 values/indices + softmax denom + the
        # combine selection stack (vector work that overlaps the scatter)
        for tt in range(NT):
            sl = slice(tt * E, (tt + 1) * E)
            nc.vector.max(out=m8_all[:, sl], in_=exp_all[:, sl])
            nc.vector.max_index(out=ei_all[:, sl], in_max=m8_all[:, sl],
                                in_values=exp_all[:, sl])
        s_all = spool.tile([P, NT], f32, name="s_all")
        nc.vector.reduce_sum(s_all[:], exp_all[:].rearrange(
            "p (t e) -> p t e", e=E), axis=mybir.AxisListType.X)
        nc.vector.reciprocal(r_all[:], s_all[:])

        # selection stack: NSEL offset planes + NSEL weight planes [P, NT],
        # plane k = 2*g + q (group-major so per-group gathers batch)
        e1f = spool.tile([P, NT], f32, name="e1f")
        e2f = spool.tile([P, NT], f32, name="e2f")
        ei3 = ei_all[:].rearrange("p (t e) -> p t e", e=E)
        nc.vector.tensor_copy(e1f[:], ei3[:, :, 0])
        nc.vector.tensor_copy(e2f[:], ei3[:, :, 1])
        ioz = spool.tile([P, NE], i32, name="ioz")
        nc.gpsimd.iota(ioz[:].rearrange("p (t e) -> p t e", e=E),
                       pattern=[[0, NT], [1, E]], base=0, channel_multiplier=0)
        iof = spool.tile([P, NE], f32, name="iof")
        nc.vector.tensor_copy(iof[:], ioz[:])
        m83 = m8_all[:].rearrange("p (t e) -> p t e", e=E)
        Ssel = spool.tile([P, 2 * NSEL, NT], f32, name="Ssel")
        for q, ef in ((0, e1f), (1, e2f)):
            oh = spool.tile([P, NE], f32, tag=f"oh{q}", name=f"oh{q}")
            nc.vector.tensor_tensor(
                out=oh[:].rearrange("p (t e) -> p t e", e=E),
                in0=iof[:].rearrange("p (t e) -> p t e", e=E),
                in1=ef[:, :, None].to_broadcast([P, NT, E]),
                op=mybir.AluOpType.is_equal)
            nc.vector.tensor_mul(oh[:], oh[:], pos_all[:])
            slot = spool.tile([P, NT], f32, tag=f"slot{q}", name=f"slot{q}")
            nc.vector.reduce_sum(slot[:], oh[:].rearrange(
                "p (t e) -> p t e", e=E), axis=mybir.AxisListType.X)
            gch = spool.tile([P, NT], f32, tag=f"gch{q}", name=f"gch{q}")
            nc.vector.tensor_scalar(gch[:], slot[:], float(CHK), None,
                                    op0=mybir.AluOpType.is_ge)
            for gg in range(2, NG):
                t2 = spool.tile([P, NT], f32, tag="t2", name="t2")
                nc.vector.tensor_scalar(t2[:], slot[:], float(CHK * gg), None,
                                        op0=mybir.AluOpType.is_ge)
                nc.vector.tensor_add(gch[:], gch[:], t2[:])
            base = spool.tile([P, NT], f32, tag=f"base{q}", name=f"base{q}")
            nc.vector.scalar_tensor_tensor(
                out=base[:], in0=ef[:], scalar=float(CHK), in1=slot[:],
                op0=mybir.AluOpType.mult, op1=mybir.AluOpType.add)
            gv = spool.tile([P, NT], f32, tag=f"gv{q}", name=f"gv{q}")
            nc.vector.tensor_tensor(out=gv[:], in0=m83[:, :, q], in1=r_all[:],
                                    op=mybir.AluOpType.mult)
            for gg in range(NG):
                k = 2 * gg + q
                eq = spool.tile([P, NT], f32, tag="eq", name="eq")
                nc.vector.tensor_scalar(eq[:], gch[:], float(gg), None,
                                        op0=mybir.AluOpType.is_equal)
                nc.vector.tensor_scalar_add(Ssel[:, k, :], base[:],
                                            float(-CHK * gg))
                nc.vector.tensor_mul(Ssel[:, k, :], Ssel[:, k, :], eq[:])
                nc.vector.tensor_mul(Ssel[:, NSEL + k, :], eq[:], gv[:])
        for j in range(OWN_TILES):
            own = ownm_sb[:, j * NT:(j + 1) * NT]
            tmpS = spool.tile([P, 2 * NSEL, NT], f32, tag="tmpS", bufs=2,
                              name="tmpS")
            nc.vector.tensor_tensor(
                out=tmpS[:], in0=Ssel[:],
                in1=own[:, None, :].to_broadcast([P, 2 * NSEL, NT]),
                op=mybir.AluOpType.mult)
            nc.vector.reduce_sum(red_sb[:, j, :], tmpS[:],
                                 axis=mybir.AxisListType.X)
            nc.vector.tensor_copy(redi_sb[:, j, :], red_sb[:, j, :NSEL])
        if dbg is not None:
            nc.sync.dma_start(
                dbg["dbg_red"][:].rearrange("p (j k) -> p j k", k=2 * NSEL),
                red_sb[:])


    # -------- FFN (bf16) + chunked y AllGather + overlapped combine --------
    with tc.tile_pool(name="ffn_sb", bufs=1, named_scope="ffn") as fpool, \
         tc.tile_pool(name="ffn_ps", bufs=1, space="PSUM") as fps, \
         tc.tile_pool(name="comb_sb", bufs=2, named_scope="combine") as cpool:
        for g in range(NG):
            t0 = g * TG
            py = [[fps.tile([P, 512], f32, tag=f"py_{t}_{n}",
                            name=f"py_{t}_{n}")
                   for n in range(2)] for t in range(TG)]
            for f in range(NF):
                ph = fps.tile([P, CHK], f32, tag="ph", bufs=1, name="ph")
                for k in range(KD):
                    nc.tensor.matmul(
                        ph[:], lhsT=w1_sb[k][:, f * P:(f + 1) * P],
                        rhs=xgT[k][:, t0 * P:t0 * P + CHK],
                        start=(k == 0), stop=(k == KD - 1))
                hbuf = fpool.tile([P, CHK], bf16, tag="hbuf", bufs=3,
                                  name="hbuf")
                nc.scalar.activation(hbuf[:], ph[:],
                                     mybir.ActivationFunctionType.Relu,
                                     bias=b1_sb[:, f:f + 1], scale=1.0)
                w2f = fpool.tile([P, D], bf16, tag="w2f", bufs=3, name="w2f")
                nc.sync.dma_start(w2f[:], w2d[f * P:(f + 1) * P, :])
                for t in range(TG):
                    for n in range(2):
                        nc.tensor.matmul(
                            py[t][n][:],
                            lhsT=hbuf[:, t * P:(t + 1) * P],
                            rhs=w2f[:, n * 512:(n + 1) * 512],
                            start=(f == 0), stop=False)
                # mid-group: transpose the NEXT group's gathered tokens into
                # xgT using the PSUM bank freed by ph bufs=1
                if f == 20 and g + 1 < NG:
                    for t in range(TG):
                        s = (g + 1) * TG + t
                        for d in range(KD):
                            pt = fps.tile([P, P], bf16, tag="ptn", bufs=1,
                                          name="ptn")
                            nc.tensor.transpose(
                                pt[:], xga[s][:, d * P:(d + 1) * P],
                                ident[:])
                            nc.vector.tensor_copy(
                                xgT[d][:, s * P:(s + 1) * P], pt[:])
            # bias b2 via rank-1 matmul, then write out
            for t in range(TG):
                for n in range(2):
                    nc.tensor.matmul(
                        py[t][n][:], lhsT=ones_row[:],
                        rhs=b2_sb[:, n * 512:(n + 1) * 512],
                        start=False, stop=True)
                ysb = fpool.tile([P, D], bf16, tag="ysb", bufs=2, name="ysb")
                nc.vector.tensor_copy(ysb[:, :512], py[t][0][:])
                nc.vector.tensor_copy(ysb[:, 512:], py[t][1][:])
                nc.sync.dma_start(y_dram[g][t * P:(t + 1) * P, :], ysb[:])
            # ship this chunk while the next group computes
            nc.gpsimd.collective_compute(
                "AllGather", mybir.AluOpType.bypass,
                replica_groups=[list(range(N_CORES))],
                ins=[y_dram[g][:].opt()],
                outs=[y_all[g][:].opt()],
            )
            if dbg is not None:
                nc.sync.dma_start(dbg["dbg_y"][g][:], y_all[g][:])
            # combine this chunk: one batched gather of all (j, q) rows,
            # then gate-weighted accumulation while the next group runs
            yt = cpool.tile([P, OWN_TILES * 2, D], bf16, tag=f"yt{g}",
                            bufs=1, name=f"yt{g}")
            for j in range(OWN_TILES):
                for q in range(2):
                    nc.gpsimd.indirect_dma_start(
                        out=yt[:, 2 * j + q, :], out_offset=None,
                        in_=y_all[g][:],
                        in_offset=IndirectOffsetOnAxis(
                            ap=redi_sb[:, j, 2 * g + q:2 * g + q + 1],
                            axis=0))
            for j in range(OWN_TILES):
                for q in range(2):
                    k = 2 * g + q
                    w = red_sb[:, j, NSEL + k:NSEL + k + 1]
                    if g == 0 and q == 0:
                        nc.vector.tensor_scalar(ot[j][:], yt[:, 2 * j + q, :],
                                                w, None,
                                                op0=mybir.AluOpType.mult)
                    else:
                        nc.vector.scalar_tensor_tensor(
                            out=ot[j][:], in0=yt[:, 2 * j + q, :], scalar=w,
                            in1=ot[j][:], op0=mybir.AluOpType.mult,
                            op1=mybir.AluOpType.add)
        for j in range(OWN_TILES):
            nc.sync.dma_start(out[j * P:(j + 1) * P, :], ot[j][:])

    persist.release()
    dram.release()


def _host_prep(x, Wg, bg, W1, b1, W2, b2, C):
    xf = np.ascontiguousarray(x.reshape(T, D).astype(np.float32))
    xT = np.ascontiguousarray(xf.T)
    xbf = xf.astype(ml_dtypes.bfloat16)
    bgb = np.tile(bg.astype(np.float32), NT)[None, :].repeat(P, 0)
    bgb = np.ascontiguousarray(bgb)
    l128 = np.triu(np.ones((P, P), np.float32), 1)  # [t', t] = 1 if t' < t
    in_maps = []
    for c in range(N_CORES):
        sel = np.zeros(E, np.float32)
        sel[c] = 1.0
        sel256 = np.ascontiguousarray(np.tile(sel, NT)[None, :].repeat(P, 0))
        ownm = np.zeros((P, OWN_TILES, NT), np.float32)
        for j in range(OWN_TILES):
            ownm[:, j, OWN_TILES * c + j] = 1.0
        in_maps.append({
            "xTf": xT,
            "xbf": xbf,
            "w1d": np.ascontiguousarray(W1[c].astype(ml_dtypes.bfloat16)),
            "w2d": np.ascontiguousarray(W2[c].astype(ml_dtypes.bfloat16)),
            "wgd": np.ascontiguousarray(Wg.astype(np.float32)),
            "bgb": bgb,
            "b1pm": np.ascontiguousarray(
                b1[c].astype(np.float32).reshape(NF, P).T),
            "b2r": np.ascontiguousarray(b2[c].astype(np.float32)[None, :]),
            "sel256": sel256,
            "l128d": l128,
            "ownmd": np.ascontiguousarray(ownm.reshape(P, OWN_TILES * NT)),
        })
    return in_maps


def _capacity(x, Wg, bg):
    xf = x.reshape(T, D).astype(np.float32)
    logits = xf @ Wg.astype(np.float32) + bg.astype(np.float32)
    part = np.partition(logits, E - 2, axis=-1)
    m2 = part[:, E - 2:E - 1]
    counts = (logits >= m2).sum(0)
    return int(np.ceil((counts.max() + 16) / CHK) * CHK)


LAST_RESULT = None


def kernel(x, Wg, bg, W1, b1, W2, b2):
    global LAST_RESULT
    from concourse.bass_utils import run_bass_kernel_spmd

    x = np.asarray(x)
    C = _capacity(x, np.asarray(Wg), np.asarray(bg))
    debug = bool(os.environ.get("BASS_DEBUG_OUT"))
    key = (C, debug)
    if key not in _cache:
        _cache[key] = build_module(C, debug_out=debug)
    nc = _cache[key]
    in_maps = _host_prep(x, np.asarray(Wg), np.asarray(bg), np.asarray(W1),
                         np.asarray(b1), np.asarray(W2), np.asarray(b2), C)
    trace = bool(os.environ.get("BASS_TRACE"))
    if trace:
        _setup_axon_profile_hook()
    res = run_bass_kernel_spmd(nc, in_maps, core_ids=list(range(N_CORES)),
                               trace=trace)
    LAST_RESULT = res
    out = np.empty((T, D), np.float32)
    for c in range(N_CORES):
        out[c * TOK_PER_CORE:(c + 1) * TOK_PER_CORE] = res.results[c]["out"]
    return out.reshape(B, S, D)


def _setup_axon_profile_hook():
    """Provide antenv.axon_hooks (missing in this image) so trace=True works."""
    import types
    try:
        import antenv
        if "antenv.axon_hooks" not in sys.modules:
            hooks = types.ModuleType("antenv.axon_hooks")
            hooks._hook = None
            hooks.set_axon_ntff_profile_hook = \
                lambda h: setattr(hooks, "_hook", h)
            hooks.get_axon_ntff_profile_hook = lambda: hooks._hook
            sys.modules["antenv.axon_hooks"] = hooks
            antenv.axon_hooks = hooks
            from trn_agent_boot.trn_boot import _ntff_profile_via_ctypes
            hooks.set_axon_ntff_profile_hook(
                _ntff_profile_via_ctypes("/opt/axon/libaxon_pjrt.so"))
    except Exception as e:  # profiling is best-effort
        print(f"profile hook setup failed: {e}", file=sys.stderr)


# revision 22
# speedup vs baseline: 1.1386x; 1.1218x over previous
"""Expert-parallel top-2 MoE kernel for 8 Trainium2 NeuronCores.

Strategy (expert-parallel, sparse dispatch, per the sharding hint):
  - Router sharded over cores: core c computes fp32 logits for its 512-token
    shard on the TensorEngine (Wg stationary) in [E, 512] layout, AllGathers
    the contiguous [E,512] blocks, and transposes to token-partition layout
    on the TensorEngine; softmax/top-2 on-device (top-2 indicator built with
    5 full-width vector ops via masked second-max).
  - Core c owns expert c. Slot positions come from matmul-based exclusive
    cumsums (single full-width matmuls). Dispatch builds the slot->token map
    with ONE multi-column indirect-DMA scatter of token ids into a [C] DRAM
    vector (tokens not routed here get an out-of-bounds slot id and are
    skipped via bounds_check), loads it back contiguously, transposes it to
    slot-tile layout on the TensorEngine, then indirect-gathers the C routed
    token rows (one op per FFN group) and transposes them into xgT.
  - Two-layer FFN in bf16 over the routed tokens (capacity padded to a
    multiple of 384) in 3 groups of 384 slots; each group's unscaled outputs
    are AllGathered right after they finish, and the combine (one batched
    indirect gather + gate-weighted fp32 accumulation) runs while the next
    group's FFN computes. Only the last chunk's AllGather is exposed.
  - Each core returns its own 512-token shard; host concatenates.

Numerics: router fp32 (top-2 selection fidelity), FFN bf16 with fp32
accumulation in PSUM, combine in fp32.
"""

import os
import sys

import numpy as np

for _p in ("/opt/trn_rl_repo",):
    if _p not in sys.path:
        sys.path.append(_p)

import ml_dtypes

import concourse.bass as bass
import concourse.mybir as mybir
import concourse.tile as tile
from concourse import bacc
from concourse.bass import IndirectOffsetOnAxis
from concourse.masks import make_identity

# Problem shapes (fixed per spec)
B, S, D, E = 2, 2048, 1024, 8
T = B * S          # 4096 tokens
F = 4 * D          # 4096 ffn dim
P = 128            # partitions
NT = T // P        # 32 token tiles
KD = D // P        # 8 contraction tiles over D
NF = F // P        # 32 f tiles
TOK_PER_CORE = T // E   # 512
OWN_TILES = TOK_PER_CORE // P  # 4
N_CORES = E
TG = 3                       # slot tiles per FFN group
CHK = TG * P                 # 384: slot chunk for the chunked AllGather
BIGP = 100000.0              # OOB slot id for tokens not routed here

f32 = mybir.dt.float32
bf16 = mybir.dt.bfloat16
i32 = mybir.dt.int32
u32 = mybir.dt.uint32

_cache = {}


def build_module(C: int, debug_out: bool = False):
    """Build the SPMD Bass module for capacity C (multiple of 384)."""
    assert C % CHK == 0
    ST = C // P  # slot tiles per expert
    NG = ST // TG

    nc = bacc.Bacc("TRN2", target_bir_lowering=False, debug=False,
                   num_devices=N_CORES)

    # ---- I/O ----
    xTf = nc.dram_tensor("xTf", [D, T], f32, kind="ExternalInput").ap()
    xbf = nc.dram_tensor("xbf", [T, D], bf16, kind="ExternalInput").ap()
    w1d = nc.dram_tensor("w1d", [D, F], bf16, kind="ExternalInput").ap()
    w2d = nc.dram_tensor("w2d", [F, D], bf16, kind="ExternalInput").ap()
    wgd = nc.dram_tensor("wgd", [D, E], f32, kind="ExternalInput").ap()
    bgb = nc.dram_tensor("bgb", [P, NT * E], f32, kind="ExternalInput").ap()
    b1pm = nc.dram_tensor("b1pm", [P, NF], f32, kind="ExternalInput").ap()
    b2r = nc.dram_tensor("b2r", [1, D], f32, kind="ExternalInput").ap()
    sel256 = nc.dram_tensor("sel256", [P, NT * E], f32,
                            kind="ExternalInput").ap()
    l128d = nc.dram_tensor("l128d", [P, P], f32, kind="ExternalInput").ap()
    ownmd = nc.dram_tensor("ownmd", [P, OWN_TILES * NT], f32,
                           kind="ExternalInput").ap()
    out = nc.dram_tensor("out", [TOK_PER_CORE, D], f32,
                         kind="ExternalOutput").ap()
    dbg = None
    if debug_out:
        dbg = {
            "dbg_l": nc.dram_tensor("dbg_l", [P, NT * E], f32,
                                    kind="ExternalOutput").ap(),
            "dbg_pos": nc.dram_tensor("dbg_pos", [P, NT * E], f32,
                                      kind="ExternalOutput").ap(),
            "dbg_ind": nc.dram_tensor("dbg_ind", [P, NT * E], f32,
                                      kind="ExternalOutput").ap(),
            "dbg_posm": nc.dram_tensor("dbg_posm", [P, NT], f32,
                                       kind="ExternalOutput").ap(),
            "dbg_idx": nc.dram_tensor("dbg_idx", [P, ST], i32,
                                      kind="ExternalOutput").ap(),
            "dbg_xgT": nc.dram_tensor("dbg_xgT", [P, C], bf16,
                                      kind="ExternalOutput").ap(),
            "dbg_red": nc.dram_tensor(
                "dbg_red", [P, OWN_TILES * 4 * NG], f32,
                kind="ExternalOutput").ap(),
            "dbg_y": [nc.dram_tensor(f"dbg_y{g}", [N_CORES * CHK, D],
                                     bf16, kind="ExternalOutput").ap()
                      for g in range(NG)],
        }

    with tile.TileContext(nc) as tc:
        _emit(tc, C, ST, xTf, xbf, w1d, w2d, wgd, bgb, b1pm, b2r, sel256,
              l128d, ownmd, out, dbg)

    nc.compile()
    return nc


def _emit(tc, C, ST, xTf, xbf, w1d, w2d, wgd, bgb, b1pm, b2r, sel256,
          l128d, ownmd, out, dbg=None):
    nc = tc.nc
    NE = NT * E  # 256
    NG = ST // TG
    NSEL = 2 * NG

    # ---------------- persistent pools ----------------
    persist = tc.alloc_tile_pool(name="persist", bufs=1)
    dram = tc.alloc_tile_pool(name="dram", bufs=1, space="DRAM")

    # tiny warmup AllGather: absorbs first-collective setup cost and aligns
    # the cores before the logits AllGather on the critical path
    wup_in = dram.tile([E, 4], f32, name="wup_in")
    wup_out = dram.tile([N_CORES * E, 4], f32, addr_space="Shared",
                        name="wup_out")
    nc.gpsimd.collective_compute(
        "AllGather", mybir.AluOpType.bypass,
        replica_groups=[list(range(N_CORES))],
        ins=[wup_in[:].opt()], outs=[wup_out[:].opt()],
    )

    # router inputs first (split across DMA queues so the router starts fast)
    wg_sb = persist.tile([P, KD, E], f32, name="wg_sb")
    nc.sync.dma_start(wg_sb[:], wgd.rearrange("(k p) e -> p k e", p=P))
    bg_sb = persist.tile([P, NE], f32, name="bg_sb")
    nc.sync.dma_start(bg_sb[:], bgb[:])
    sel_sb = persist.tile([P, NE], f32, name="sel_sb")
    nc.sync.dma_start(sel_sb[:], sel256[:])
    l128_sb = persist.tile([P, P], f32, name="l128_sb")
    nc.sync.dma_start(l128_sb[:], l128d[:])
    ownm_sb = persist.tile([P, OWN_TILES * NT], f32, name="ownm_sb")
    nc.sync.dma_start(ownm_sb[:], ownmd[:])
    b1_sb = persist.tile([P, NF], f32, name="b1_sb")
    nc.sync.dma_start(b1_sb[:], b1pm[:])
    b2_sb = persist.tile([1, D], f32, name="b2_sb")
    nc.sync.dma_start(b2_sb[:], b2r[:])
    ident = persist.tile([P, P], bf16, name="ident")
    make_identity(nc, ident[:])
    identf = persist.tile([P, P], f32, name="identf")
    make_identity(nc, identf[:])
    ones_col = persist.tile([P, 1], f32, name="ones_col")
    nc.vector.memset(ones_col[:], 1.0)
    ones_row = persist.tile([1, P], f32, name="ones_row")
    nc.vector.memset(ones_row[:], 1.0)
    # slot iota (u16: 2x DVE rate) and the [p, tile] pair for idx matmuls
    iotaC = persist.tile([P, C], mybir.dt.uint16, name="iotaC")
    iotaC_i = persist.tile([P, C], i32, name="iotaC_i")
    nc.gpsimd.iota(iotaC_i[:], pattern=[[1, C]], base=0, channel_multiplier=0)
    nc.vector.tensor_copy(iotaC[:], iotaC_i[:])
    pv2 = persist.tile([P, NT, 2], bf16, name="pv2")
    pv2_i = persist.tile([P, NT, 2], i32, name="pv2_i")
    nc.gpsimd.iota(pv2_i[:, :, 0], pattern=[[0, NT]], base=0,
                   channel_multiplier=1)
    nc.gpsimd.iota(pv2_i[:, :, 1], pattern=[[1, NT]], base=0,
                   channel_multiplier=0)
    nc.vector.tensor_copy(pv2[:], pv2_i[:])

    # router / dispatch state kept for the combine phase
    exp_all = persist.tile([P, NE], f32, name="exp_all")    # exp(logits)
    m8_all = persist.tile([P, NE], f32, name="m8_all")      # per-tile top8
    r_all = persist.tile([P, NT], f32, name="r_all")        # 1/sum(exp)
    pos_all = persist.tile([P, NE], f32, name="pos_all")    # excl cumsum
    ind_all = persist.tile([P, NE], f32, name="ind_all")    # top2 indicator
    ei_all = persist.tile([P, NE], u32, name="ei_all")      # top8 indices
    # selection data, plane index k = 2*g + q (group-major for batched gather)
    red_sb = persist.tile([P, OWN_TILES, 2 * NSEL], f32, name="red_sb")
    redi_sb = persist.tile([P, OWN_TILES, NSEL], i32, name="redi_sb")
    ot = [persist.tile([P, D], f32, name=f"ot{j}") for j in range(OWN_TILES)]

    xgT = [persist.tile([P, C], bf16, name=f"xgT{d}") for d in range(KD)]

    y_dram = [dram.tile([CHK, D], bf16, name=f"y_dram{g}") for g in range(NG)]
    y_all = [dram.tile([N_CORES * CHK, D], bf16, addr_space="Shared",
                       name=f"y_all{g}") for g in range(NG)]

    # ------------- router: every core computes ALL logits locally -------------
    # (full xT fp32 is an input; no logits collective on the critical path)
    with tc.tile_pool(name="router_sb", bufs=1, named_scope="router") as rpool, \
         tc.tile_pool(name="router_ps", bufs=1, space="PSUM") as rps:
        l_all = rpool.tile([P, NE], f32, name="l_all")
        xTv = xTf.rearrange("(k p) t -> p k t", p=P)
        NCH = T // TOK_PER_CORE  # 8 chunks of 512 tokens
        QT = TOK_PER_CORE // P   # 4 token tiles per chunk
        for c in range(NCH):
            xsc = rpool.tile([P, KD, TOK_PER_CORE], f32, tag="xsc", bufs=3,
                             name="xsc")
            for k in range(KD):
                nc.sync.dma_start(
                    xsc[:, k, :],
                    xTv[:, k, c * TOK_PER_CORE:(c + 1) * TOK_PER_CORE])
            lT = rps.tile([E, TOK_PER_CORE], f32, tag="lT", bufs=2, name="lT")
            for k in range(KD):
                nc.tensor.matmul(lT[:], lhsT=wg_sb[:, k, :], rhs=xsc[:, k, :],
                                 start=(k == 0), stop=(k == KD - 1))
            lsb = rpool.tile([E, TOK_PER_CORE], f32, tag="lsb", bufs=2,
                             name="lsb")
            nc.vector.tensor_copy(lsb[:], lT[:])
            pt_l = rps.tile([P, QT, E], f32, tag="pt_l", bufs=2, name="pt_l")
            for q in range(QT):
                nc.tensor.transpose(
                    pt_l[:, q, :], lsb[:, q * P:(q + 1) * P], identf[:E, :E])
            nc.vector.tensor_copy(
                l_all[:, c * QT * E:(c + 1) * QT * E], pt_l[:])
        nc.vector.tensor_add(l_all[:], l_all[:], bg_sb[:])
        if dbg is not None:
            nc.sync.dma_start(dbg["dbg_l"][:], l_all[:])
        nc.scalar.activation(exp_all[:], l_all[:],
                             mybir.ActivationFunctionType.Exp)
        # top-2 indicator via masked second-max (5 full-width ops)
        exp3 = exp_all[:].rearrange("p (t e) -> p t e", e=E)
        mx = rpool.tile([P, NT], f32, name="mx")
        nc.vector.reduce_max(mx[:], exp3, axis=mybir.AxisListType.X)
        eqm = rpool.tile([P, NE], f32, name="eqm")
        nc.vector.tensor_tensor(
            out=eqm[:].rearrange("p (t e) -> p t e", e=E), in0=exp3,
            in1=mx[:, :, None].to_broadcast([P, NT, E]),
            op=mybir.AluOpType.is_ge)
        nm = rpool.tile([P, NE], f32, name="nm")
        nc.vector.scalar_tensor_tensor(
            out=nm[:], in0=eqm[:], scalar=-BIGP, in1=exp_all[:],
            op0=mybir.AluOpType.mult, op1=mybir.AluOpType.add)
        m2 = rpool.tile([P, NT], f32, name="m2")
        nc.vector.reduce_max(m2[:], nm[:].rearrange("p (t e) -> p t e", e=E),
                             axis=mybir.AxisListType.X)
        nc.vector.tensor_tensor(
            out=ind_all[:].rearrange("p (t e) -> p t e", e=E), in0=exp3,
            in1=m2[:, :, None].to_broadcast([P, NT, E]),
            op=mybir.AluOpType.is_ge)

    # expert weights: queued after the router's xT chunks so they don't
    # pace the router; done well before the FFN needs them
    w1_sb = [persist.tile([P, F], bf16, name=f"w1_sb{k}") for k in range(KD)]
    for k in range(KD):
        nc.sync.dma_start(w1_sb[k][:], w1d[k * P:(k + 1) * P, :])

    # ---------------- dispatch: cumsum positions + id scatter ----------------
    with tc.tile_pool(name="disp_sb", bufs=1, named_scope="dispatch") as dpool, \
         tc.tile_pool(name="disp_ps", bufs=1, space="PSUM") as dps:
        # per-(tile,expert) totals in one matmul
        ptot = dps.tile([1, NE], f32, name="ptot")
        nc.tensor.matmul(ptot[:], lhsT=ones_col[:], rhs=ind_all[:],
                         start=True, stop=True)
        tot_flat = dpool.tile([1, NE], f32, name="tot_flat")
        nc.vector.tensor_copy(tot_flat[:], ptot[:])
        # reshape [1, NT*E] -> [NT, E] via sbuf-to-sbuf DMA
        tot32 = dpool.tile([NT, E], f32, name="tot32")
        nc.sync.dma_start(tot32[:], tot_flat[:])
        # exclusive cumsum over tiles: strict-lower matmul
        pofs = dps.tile([NT, E], f32, name="pofs")
        nc.tensor.matmul(pofs[:], lhsT=l128_sb[:NT, :NT], rhs=tot32[:],
                         start=True, stop=True)
        ofs32 = dpool.tile([NT, E], f32, name="ofs32")
        nc.vector.tensor_copy(ofs32[:], pofs[:])
        ofs_flat = dpool.tile([1, NE], f32, name="ofs_flat")
        nc.sync.dma_start(ofs_flat[:], ofs32[:])
        # positions: local excl cumsum + broadcast tile offset, one pass
        ppos = dps.tile([P, NE], f32, name="ppos")
        nc.tensor.matmul(ppos[:], lhsT=l128_sb[:], rhs=ind_all[:],
                         start=True, stop=False)
        nc.tensor.matmul(ppos[:], lhsT=ones_row[:], rhs=ofs_flat[:],
                         start=False, stop=True)
        nc.vector.tensor_copy(pos_all[:], ppos[:])

        # my expert's masked positions: ind ? pos : BIGP (skipped as OOB)
        tmp = dpool.tile([P, NE], f32, name="tmp")
        nc.vector.tensor_scalar_add(tmp[:], pos_all[:], -BIGP)
        nc.vector.tensor_mul(tmp[:], tmp[:], ind_all[:])
        nc.vector.tensor_mul(tmp[:], tmp[:], sel_sb[:])
        pos_m = dpool.tile([P, NT], f32, name="pos_m")
        nc.vector.reduce_sum(pos_m[:], tmp[:].rearrange(
            "p (t e) -> p t e", e=E), axis=mybir.AxisListType.X)
        nc.vector.tensor_scalar_add(pos_m[:], pos_m[:], BIGP)
        if dbg is not None:
            nc.sync.dma_start(dbg["dbg_pos"][:], pos_all[:])
            nc.sync.dma_start(dbg["dbg_ind"][:], ind_all[:])
            nc.sync.dma_start(dbg["dbg_posm"][:], pos_m[:])

        # slot->token map via one-hot matmuls, built group by group so the
        # first FFN group's gather can start as early as possible:
        # Pt[p, s] = (pos_m[p,t]==s); acc[0,s] = partition, acc[1,s] = tile
        idx2_sb = dpool.tile([2, C], f32, name="idx2_sb")
        for g in range(NG):
            acc = dps.tile([2, CHK], f32, tag=f"accx{g}", name=f"accx{g}")
            for tt in range(NT):
                Pt = dpool.tile([P, CHK], bf16, tag="Pt", bufs=4, name="Pt")
                nc.vector.tensor_scalar(
                    Pt[:], iotaC[:, g * CHK:(g + 1) * CHK],
                    pos_m[:, tt:tt + 1], None,
                    op0=mybir.AluOpType.is_equal)
                nc.tensor.matmul(acc[:], lhsT=pv2[:, tt, :], rhs=Pt[:],
                                 start=(tt == 0), stop=(tt == NT - 1))
            nc.vector.tensor_copy(idx2_sb[:, g * CHK:(g + 1) * CHK], acc[:])

    # ---- finish idx per group (transpose + fold tile id) + gathers ----
    # xg tiles must outlive this scope (group>0 transposes run inside the
    # FFN section), so they live in a dedicated pool released after FFN.
    xga = [persist.tile([P, D], bf16, name=f"xga{s}") for s in range(ST)]
    with tc.tile_pool(name="gat_sb", bufs=2) as gpool, \
         tc.tile_pool(name="gat_ps", bufs=2, space="PSUM") as gps:
        idx_i = gpool.tile([P, ST], i32, bufs=1, name="idx_i")
        for g in range(NG):
            gsl = slice(g * TG, (g + 1) * TG)
            pti = gps.tile([P, TG, 2], f32, tag="pti", bufs=2, name="pti")
            for t in range(TG):
                s = g * TG + t
                nc.tensor.transpose(pti[:, t, :],
                                    idx2_sb[:, s * P:(s + 1) * P],
                                    identf[:2, :2])
            pti_sb = gpool.tile([P, TG, 2], f32, tag="pti_sb", bufs=2,
                                name="pti_sb")
            nc.vector.tensor_copy(pti_sb[:], pti[:])
            idx_f = gpool.tile([P, TG], f32, tag="idx_f", bufs=2,
                               name="idx_f")
            nc.vector.scalar_tensor_tensor(
                out=idx_f[:], in0=pti_sb[:, :, 1], scalar=float(P),
                in1=pti_sb[:, :, 0], op0=mybir.AluOpType.mult,
                op1=mybir.AluOpType.add)
            nc.vector.tensor_copy(idx_i[:, gsl], idx_f[:])
            for t in range(TG):
                s = g * TG + t
                nc.gpsimd.indirect_dma_start(
                    out=xga[s][:], out_offset=None, in_=xbf[:],
                    in_offset=IndirectOffsetOnAxis(ap=idx_i[:, s:s + 1],
                                                   axis=0),
                )
        if dbg is not None:
            nc.sync.dma_start(dbg["dbg_idx"][:], idx_i[:])
        # transpose all gathered slot tiles into xgT (pre-FFN; full-rate FFN
        # beats overlapping these into the FFN's PSUM budget)
        for s in range(ST):
            for d in range(KD):
                pt = gps.tile([P, P], bf16, tag="pt", name="pt")
                nc.tensor.transpose(pt[:], xga[s][:, d * P:(d + 1) * P],
                                    ident[:])
                nc.vector.tensor_copy(xgT[d][:, s * P:(s + 1) * P], pt[:])

    with tc.tile_pool(name="sel_sb", bufs=1, named_scope="select") as spool:
        # off the critical path: top-8 values/indices + softmax denom + the
        # combine selection stack (vector work that overlaps the scatter)
        for tt in range(NT):
            sl = slice(tt * E, (tt + 1) * E)
            nc.vector.max(out=m8_all[:, sl], in_=exp_all[:, sl])
            nc.vector.max_index(out=ei_all[:, sl], in_max=m8_all[:, sl],
                                in_values=exp_all[:, sl])
        s_all = spool.tile([P, NT], f32, name="s_all")
        nc.vector.reduce_sum(s_all[:], exp_all[:].rearrange(
            "p (t e) -> p t e", e=E), axis=mybir.AxisListType.X)
        nc.vector.reciprocal(r_all[:], s_all[:])

        # selection stack: NSEL offset planes + NSEL weight planes [P, NT],
        # plane k = 2*g + q (group-major so per-group gathers batch)
        e1f = spool.tile([P, NT], f32, name="e1f")
        e2f = spool.tile([P, NT], f32, name="e2f")
        ei3 = ei_all[:].rearrange("p (t e) -> p t e", e=E)
        nc.vector.tensor_copy(e1f[:], ei3[:, :, 0])
        nc.vector.tensor_copy(e2f[:], ei3[:, :, 1])
        ioz = spool.tile([P, NE], i32, name="ioz")
        nc.gpsimd.iota(ioz[:].rearrange("p (t e) -> p t e", e=E),
                       pattern=[[0, NT], [1, E]], base=0, channel_multiplier=0)
        iof = spool.tile([P, NE], f32, name="iof")
        nc.vector.tensor_copy(iof[:], ioz[:])
        m83 = m8_all[:].rearrange("p (t e) -> p t e", e=E)
        Ssel = spool.tile([P, 2 * NSEL, NT], f32, name="Ssel")
        for q, ef in ((0, e1f), (1, e2f)):
            oh = spool.tile([P, NE], f32, tag=f"oh{q}", name=f"oh{q}")
            nc.vector.tensor_tensor(
                out=oh[:].rearrange("p (t e) -> p t e", e=E),
                in0=iof[:].rearrange("p (t e) -> p t e", e=E),
                in1=ef[:, :, None].to_broadcast([P, NT, E]),
                op=mybir.AluOpType.is_equal)
            nc.vector.tensor_mul(oh[:], oh[:], pos_all[:])
            slot = spool.tile([P, NT], f32, tag=f"slot{q}", name=f"slot{q}")
            nc.vector.reduce_sum(slot[:], oh[:].rearrange(
                "p (t e) -> p t e", e=E), axis=mybir.AxisListType.X)
            gch = spool.tile([P, NT], f32, tag=f"gch{q}", name=f"gch{q}")
            nc.vector.tensor_scalar(gch[:], slot[:], float(CHK), None,
                                    op0=mybir.AluOpType.is_ge)
            for gg in range(2, NG):
                t2 = spool.tile([P, NT], f32, tag="t2", name="t2")
                nc.vector.tensor_scalar(t2[:], slot[:], float(CHK * gg), None,
                                        op0=mybir.AluOpType.is_ge)
                nc.vector.tensor_add(gch[:], gch[:], t2[:])
            base = spool.tile([P, NT], f32, tag=f"base{q}", name=f"base{q}")
            nc.vector.scalar_tensor_tensor(
                out=base[:], in0=ef[:], scalar=float(CHK), in1=slot[:],
                op0=mybir.AluOpType.mult, op1=mybir.AluOpType.add)
            gv = spool.tile([P, NT], f32, tag=f"gv{q}", name=f"gv{q}")
            nc.vector.tensor_tensor(out=gv[:], in0=m83[:, :, q], in1=r_all[:],
                                    op=mybir.AluOpType.mult)
            for gg in range(NG):
                k = 2 * gg + q
                eq = spool.tile([P, NT], f32, tag="eq", name="eq")
                nc.vector.tensor_scalar(eq[:], gch[:], float(gg), None,
                                        op0=mybir.AluOpType.is_equal)
                nc.vector.tensor_scalar_add(Ssel[:, k, :], base[:],
                                            float(-CHK * gg))
                nc.vector.tensor_mul(Ssel[:, k, :], Ssel[:, k, :], eq[:])
                nc.vector.tensor_mul(Ssel[:, NSEL + k, :], eq[:], gv[:])
        for j in range(OWN_TILES):
            own = ownm_sb[:, j * NT:(j + 1) * NT]
            tmpS = spool.tile([P, 2 * NSEL, NT], f32, tag="tmpS", bufs=2,
                              name="tmpS")
            nc.vector.tensor_tensor(
                out=tmpS[:], in0=Ssel[:],
                in1=own[:, None, :].to_broadcast([P, 2 * NSEL, NT]),
                op=mybir.AluOpType.mult)
            nc.vector.reduce_sum(red_sb[:, j, :], tmpS[:],
                                 axis=mybir.AxisListType.X)
            nc.vector.tensor_copy(redi_sb[:, j, :], red_sb[:, j, :NSEL])
        if dbg is not None:
            nc.sync.dma_start(
                dbg["dbg_red"][:].rearrange("p (j k) -> p j k", k=2 * NSEL),
                red_sb[:])


    # -------- FFN (bf16) + chunked y AllGather + overlapped combine --------
    with tc.tile_pool(name="ffn_sb", bufs=1, named_scope="ffn") as fpool, \
         tc.tile_pool(name="ffn_ps", bufs=1, space="PSUM") as fps, \
         tc.tile_pool(name="comb_sb", bufs=2, named_scope="combine") as cpool:
        for g in range(NG):
            t0 = g * TG
            py = [[fps.tile([P, 512], f32, tag=f"py_{t}_{n}",
                            name=f"py_{t}_{n}")
                   for n in range(2)] for t in range(TG)]
            for f in range(NF):
                ph = fps.tile([P, CHK], f32, tag="ph", bufs=2, name="ph")
                for k in range(KD):
                    nc.tensor.matmul(
                        ph[:], lhsT=w1_sb[k][:, f * P:(f + 1) * P],
                        rhs=xgT[k][:, t0 * P:t0 * P + CHK],
                        start=(k == 0), stop=(k == KD - 1))
                hbuf = fpool.tile([P, CHK], bf16, tag="hbuf", bufs=3,
                                  name="hbuf")
                nc.scalar.activation(hbuf[:], ph[:],
                                     mybir.ActivationFunctionType.Relu,
                                     bias=b1_sb[:, f:f + 1], scale=1.0)
                w2f = fpool.tile([P, D], bf16, tag="w2f", bufs=3, name="w2f")
                nc.sync.dma_start(w2f[:], w2d[f * P:(f + 1) * P, :])
                for t in range(TG):
                    for n in range(2):
                        nc.tensor.matmul(
                            py[t][n][:],
                            lhsT=hbuf[:, t * P:(t + 1) * P],
                            rhs=w2f[:, n * 512:(n + 1) * 512],
                            start=(f == 0), stop=False)
            # bias b2 via rank-1 matmul, then write out
            for t in range(TG):
                for n in range(2):
                    nc.tensor.matmul(
                        py[t][n][:], lhsT=ones_row[:],
                        rhs=b2_sb[:, n * 512:(n + 1) * 512],
                        start=False, stop=True)
                ysb = fpool.tile([P, D], bf16, tag="ysb", bufs=2, name="ysb")
                nc.vector.tensor_copy(ysb[:, :512], py[t][0][:])
                nc.vector.tensor_copy(ysb[:, 512:], py[t][1][:])
                nc.sync.dma_start(y_dram[g][t * P:(t + 1) * P, :], ysb[:])
            # ship this chunk while the next group computes
            nc.gpsimd.collective_compute(
                "AllGather", mybir.AluOpType.bypass,
                replica_groups=[list(range(N_CORES))],
                ins=[y_dram[g][:].opt()],
                outs=[y_all[g][:].opt()],
            )
            if dbg is not None:
                nc.sync.dma_start(dbg["dbg_y"][g][:], y_all[g][:])
            # combine this chunk: one batched gather of all (j, q) rows,
            # then gate-weighted accumulation while the next group runs
            yt = cpool.tile([P, OWN_TILES * 2, D], bf16, tag=f"yt{g}",
                            bufs=1, name=f"yt{g}")
            for j in range(OWN_TILES):
                for q in range(2):
                    nc.gpsimd.indirect_dma_start(
                        out=yt[:, 2 * j + q, :], out_offset=None,
                        in_=y_all[g][:],
                        in_offset=IndirectOffsetOnAxis(
                            ap=redi_sb[:, j, 2 * g + q:2 * g + q + 1],
                            axis=0))
            for j in range(OWN_TILES):
                for q in range(2):
                    k = 2 * g + q
                    w = red_sb[:, j, NSEL + k:NSEL + k + 1]
                    if g == 0 and q == 0:
                        nc.vector.tensor_scalar(ot[j][:], yt[:, 2 * j + q, :],
                                                w, None,
                                                op0=mybir.AluOpType.mult)
                    else:
                        nc.vector.scalar_tensor_tensor(
                            out=ot[j][:], in0=yt[:, 2 * j + q, :], scalar=w,
                            in1=ot[j][:], op0=mybir.AluOpType.mult,
                            op1=mybir.AluOpType.add)
        for j in range(OWN_TILES):
            nc.sync.dma_start(out[j * P:(j + 1) * P, :], ot[j][:])

    persist.release()
    dram.release()


def _host_prep(x, Wg, bg, W1, b1, W2, b2, C):
    xf = np.ascontiguousarray(x.reshape(T, D).astype(np.float32))
    xT = np.ascontiguousarray(xf.T)
    xbf = xf.astype(ml_dtypes.bfloat16)
    bgb = np.tile(bg.astype(np.float32), NT)[None, :].repeat(P, 0)
    bgb = np.ascontiguousarray(bgb)
    l128 = np.triu(np.ones((P, P), np.float32), 1)  # [t', t] = 1 if t' < t
    in_maps = []
    for c in range(N_CORES):
        sel = np.zeros(E, np.float32)
        sel[c] = 1.0
        sel256 = np.ascontiguousarray(np.tile(sel, NT)[None, :].repeat(P, 0))
        ownm = np.zeros((P, OWN_TILES, NT), np.float32)
        for j in range(OWN_TILES):
            ownm[:, j, OWN_TILES * c + j] = 1.0
        in_maps.append({
            "xTf": xT,
            "xbf": xbf,
            "w1d": np.ascontiguousarray(W1[c].astype(ml_dtypes.bfloat16)),
            "w2d": np.ascontiguousarray(W2[c].astype(ml_dtypes.bfloat16)),
            "wgd": np.ascontiguousarray(Wg.astype(np.float32)),
            "bgb": bgb,
            "b1pm": np.ascontiguousarray(
                b1[c].astype(np.float32).reshape(NF, P).T),
            "b2r": np.ascontiguousarray(b2[c].astype(np.float32)[None, :]),
            "sel256": sel256,
            "l128d": l128,
            "ownmd": np.ascontiguousarray(ownm.reshape(P, OWN_TILES * NT)),
        })
    return in_maps


def _capacity(x, Wg, bg):
    xf = x.reshape(T, D).astype(np.float32)
    logits = xf @ Wg.astype(np.float32) + bg.astype(np.float32)
    part = np.partition(logits, E - 2, axis=-1)
    m2 = part[:, E - 2:E - 1]
    counts = (logits >= m2).sum(0)
    return int(np.ceil((counts.max() + 16) / CHK) * CHK)


LAST_RESULT = None


def kernel(x, Wg, bg, W1, b1, W2, b2):
    global LAST_RESULT
    from concourse.bass_utils import run_bass_kernel_spmd

    x = np.asarray(x)
    C = _capacity(x, np.asarray(Wg), np.asarray(bg))
    debug = bool(os.environ.get("BASS_DEBUG_OUT"))
    key = (C, debug)
    if key not in _cache:
        _cache[key] = build_module(C, debug_out=debug)
    nc = _cache[key]
    in_maps = _host_prep(x, np.asarray(Wg), np.asarray(bg), np.asarray(W1),
                         np.asarray(b1), np.asarray(W2), np.asarray(b2), C)
    trace = bool(os.environ.get("BASS_TRACE"))
    if trace:
        _setup_axon_profile_hook()
    res = run_bass_kernel_spmd(nc, in_maps, core_ids=list(range(N_CORES)),
                               trace=trace)
    LAST_RESULT = res
    out = np.empty((T, D), np.float32)
    for c in range(N_CORES):
        out[c * TOK_PER_CORE:(c + 1) * TOK_PER_CORE] = res.results[c]["out"]
    return out.reshape(B, S, D)


def _setup_axon_profile_hook():
    """Provide antenv.axon_hooks (missing in this image) so trace=True works."""
    import types
    try:
        import antenv
        if "antenv.axon_hooks" not in sys.modules:
            hooks = types.ModuleType("antenv.axon_hooks")
            hooks._hook = None
            hooks.set_axon_ntff_profile_hook = \
                lambda h: setattr(hooks, "_hook", h)
            hooks.get_axon_ntff_profile_hook = lambda: hooks._hook
            sys.modules["antenv.axon_hooks"] = hooks
            antenv.axon_hooks = hooks
            from trn_agent_boot.trn_boot import _ntff_profile_via_ctypes
            hooks.set_axon_ntff_profile_hook(
                _ntff_profile_via_ctypes("/opt/axon/libaxon_pjrt.so"))
    except Exception as e:  # profiling is best-effort
        print(f"profile hook setup failed: {e}", file=sys.stderr)


# revision 27
# speedup vs baseline: 1.1680x; 1.0259x over previous
"""Expert-parallel top-2 MoE kernel for 8 Trainium2 NeuronCores.

Strategy (expert-parallel, sparse dispatch, per the sharding hint):
  - Router sharded over cores: core c computes fp32 logits for its 512-token
    shard on the TensorEngine (Wg stationary) in [E, 512] layout, AllGathers
    the contiguous [E,512] blocks, and transposes to token-partition layout
    on the TensorEngine; softmax/top-2 on-device (top-2 indicator built with
    5 full-width vector ops via masked second-max).
  - Core c owns expert c. Slot positions come from matmul-based exclusive
    cumsums (single full-width matmuls). Dispatch builds the slot->token map
    with ONE multi-column indirect-DMA scatter of token ids into a [C] DRAM
    vector (tokens not routed here get an out-of-bounds slot id and are
    skipped via bounds_check), loads it back contiguously, transposes it to
    slot-tile layout on the TensorEngine, then indirect-gathers the C routed
    token rows (one op per FFN group) and transposes them into xgT.
  - Two-layer FFN in bf16 over the routed tokens (capacity padded to a
    multiple of 384) in 3 groups of 384 slots; each group's unscaled outputs
    are AllGathered right after they finish, and the combine (one batched
    indirect gather + gate-weighted fp32 accumulation) runs while the next
    group's FFN computes. Only the last chunk's AllGather is exposed.
  - Each core returns its own 512-token shard; host concatenates.

Numerics: router fp32 (top-2 selection fidelity), FFN bf16 with fp32
accumulation in PSUM, combine in fp32.
"""

import os
import sys

import numpy as np

for _p in ("/opt/trn_rl_repo",):
    if _p not in sys.path:
        sys.path.append(_p)

import ml_dtypes

import concourse.bass as bass
import concourse.mybir as mybir
import concourse.tile as tile
from concourse import bacc
from concourse.bass import IndirectOffsetOnAxis
from concourse.masks import make_identity

# Problem shapes (fixed per spec)
B, S, D, E = 2, 2048, 1024, 8
T = B * S          # 4096 tokens
F = 4 * D          # 4096 ffn dim
P = 128            # partitions
NT = T // P        # 32 token tiles
KD = D // P        # 8 contraction tiles over D
NF = F // P        # 32 f tiles
TOK_PER_CORE = T // E   # 512
OWN_TILES = TOK_PER_CORE // P  # 4
N_CORES = E
TG = 3                       # slot tiles per FFN group
CHK = TG * P                 # 384: slot chunk for the chunked AllGather
BIGP = 100000.0              # OOB slot id for tokens not routed here

f32 = mybir.dt.float32
bf16 = mybir.dt.bfloat16
i32 = mybir.dt.int32
u32 = mybir.dt.uint32

_cache = {}


def build_module(C: int, debug_out: bool = False):
    """Build the SPMD Bass module for capacity C (multiple of 384)."""
    assert C % CHK == 0
    ST = C // P  # slot tiles per expert
    NG = ST // TG

    nc = bacc.Bacc("TRN2", target_bir_lowering=False, debug=False,
                   num_devices=N_CORES)

    # ---- I/O ----
    xTf = nc.dram_tensor("xTf", [D, T], f32, kind="ExternalInput").ap()
    xbf = nc.dram_tensor("xbf", [T, D], bf16, kind="ExternalInput").ap()
    w1d = nc.dram_tensor("w1d", [D, F], bf16, kind="ExternalInput").ap()
    w2d = nc.dram_tensor("w2d", [F, D], bf16, kind="ExternalInput").ap()
    wgd = nc.dram_tensor("wgd", [D, E], f32, kind="ExternalInput").ap()
    bgb = nc.dram_tensor("bgb", [P, NT * E], f32, kind="ExternalInput").ap()
    b1pm = nc.dram_tensor("b1pm", [P, NF], f32, kind="ExternalInput").ap()
    b2r = nc.dram_tensor("b2r", [1, D], f32, kind="ExternalInput").ap()
    sel256 = nc.dram_tensor("sel256", [P, NT * E], f32,
                            kind="ExternalInput").ap()
    selEd = nc.dram_tensor("selEd", [E, P], f32, kind="ExternalInput").ap()
    l128d = nc.dram_tensor("l128d", [P, P], f32, kind="ExternalInput").ap()
    ownmd = nc.dram_tensor("ownmd", [P, OWN_TILES * NT], f32,
                           kind="ExternalInput").ap()
    out = nc.dram_tensor("out", [TOK_PER_CORE, D], f32,
                         kind="ExternalOutput").ap()
    dbg = None
    if debug_out:
        dbg = {
            "dbg_l": nc.dram_tensor("dbg_l", [P, NT * E], f32,
                                    kind="ExternalOutput").ap(),
            "dbg_pos": nc.dram_tensor("dbg_pos", [P, NT * E], f32,
                                      kind="ExternalOutput").ap(),
            "dbg_ind": nc.dram_tensor("dbg_ind", [P, NT * E], f32,
                                      kind="ExternalOutput").ap(),
            "dbg_posm": nc.dram_tensor("dbg_posm", [P, NT], f32,
                                       kind="ExternalOutput").ap(),
            "dbg_idx": nc.dram_tensor("dbg_idx", [P, ST], i32,
                                      kind="ExternalOutput").ap(),
            "dbg_xgT": nc.dram_tensor("dbg_xgT", [P, C], bf16,
                                      kind="ExternalOutput").ap(),
            "dbg_red": nc.dram_tensor(
                "dbg_red", [P, OWN_TILES * 4 * NG], f32,
                kind="ExternalOutput").ap(),
            "dbg_y": [nc.dram_tensor(f"dbg_y{g}", [N_CORES * CHK, D],
                                     bf16, kind="ExternalOutput").ap()
                      for g in range(NG)],
        }

    with tile.TileContext(nc) as tc:
        _emit(tc, C, ST, xTf, xbf, w1d, w2d, wgd, bgb, b1pm, b2r, sel256,
              selEd, l128d, ownmd, out, dbg)

    nc.compile()
    return nc


def _emit(tc, C, ST, xTf, xbf, w1d, w2d, wgd, bgb, b1pm, b2r, sel256,
          selEd, l128d, ownmd, out, dbg=None):
    nc = tc.nc
    NE = NT * E  # 256
    NG = ST // TG
    NSEL = 2 * NG

    # ---------------- persistent pools ----------------
    persist = tc.alloc_tile_pool(name="persist", bufs=1)
    dram = tc.alloc_tile_pool(name="dram", bufs=1, space="DRAM")

    # tiny warmup AllGather: absorbs first-collective setup cost and aligns
    # the cores before the logits AllGather on the critical path
    wup_in = dram.tile([E, 4], f32, name="wup_in")
    wup_out = dram.tile([N_CORES * E, 4], f32, addr_space="Shared",
                        name="wup_out")
    nc.gpsimd.collective_compute(
        "AllGather", mybir.AluOpType.bypass,
        replica_groups=[list(range(N_CORES))],
        ins=[wup_in[:].opt()], outs=[wup_out[:].opt()],
    )

    # router inputs first (split across DMA queues so the router starts fast)
    wg_sb = persist.tile([P, KD, E], f32, name="wg_sb")
    nc.sync.dma_start(wg_sb[:], wgd.rearrange("(k p) e -> p k e", p=P))
    bg_sb = persist.tile([P, NE], f32, name="bg_sb")
    nc.sync.dma_start(bg_sb[:], bgb[:])
    sel_sb = persist.tile([P, NE], f32, name="sel_sb")
    nc.sync.dma_start(sel_sb[:], sel256[:])
    selE_sb = persist.tile([E, P], f32, name="selE_sb")
    nc.sync.dma_start(selE_sb[:], selEd[:])
    l128_sb = persist.tile([P, P], f32, name="l128_sb")
    nc.sync.dma_start(l128_sb[:], l128d[:])
    ownm_sb = persist.tile([P, OWN_TILES * NT], f32, name="ownm_sb")
    nc.sync.dma_start(ownm_sb[:], ownmd[:])
    b1_sb = persist.tile([P, NF], f32, name="b1_sb")
    nc.sync.dma_start(b1_sb[:], b1pm[:])
    b2_sb = persist.tile([1, D], f32, name="b2_sb")
    nc.sync.dma_start(b2_sb[:], b2r[:])
    ident = persist.tile([P, P], bf16, name="ident")
    make_identity(nc, ident[:])
    identf = persist.tile([P, P], f32, name="identf")
    make_identity(nc, identf[:])
    ones_col = persist.tile([P, 1], f32, name="ones_col")
    nc.vector.memset(ones_col[:], 1.0)
    ones_row = persist.tile([1, P], f32, name="ones_row")
    nc.vector.memset(ones_row[:], 1.0)
    # slot iota (u16: 2x DVE rate) and the [p, tile] pair for idx matmuls
    iotaC = persist.tile([P, C], mybir.dt.uint16, name="iotaC")
    iotaC_i = persist.tile([P, C], i32, name="iotaC_i")
    nc.gpsimd.iota(iotaC_i[:], pattern=[[1, C]], base=0, channel_multiplier=0)
    nc.vector.tensor_copy(iotaC[:], iotaC_i[:])
    pv2 = persist.tile([P, NT, 2], bf16, name="pv2")
    pv2_i = persist.tile([P, NT, 2], i32, name="pv2_i")
    nc.gpsimd.iota(pv2_i[:, :, 0], pattern=[[0, NT]], base=0,
                   channel_multiplier=1)
    nc.gpsimd.iota(pv2_i[:, :, 1], pattern=[[1, NT]], base=0,
                   channel_multiplier=0)
    nc.vector.tensor_copy(pv2[:], pv2_i[:])

    # router / dispatch state kept for the combine phase
    exp_all = persist.tile([P, NE], f32, name="exp_all")    # exp(logits)
    m8_all = persist.tile([P, NE], f32, name="m8_all")      # per-tile top8
    r_all = persist.tile([P, NT], f32, name="r_all")        # 1/sum(exp)
    pos_all = persist.tile([P, NE], f32, name="pos_all")    # excl cumsum
    ind_all = persist.tile([P, NE], f32, name="ind_all")    # top2 indicator
    ei_all = persist.tile([P, NE], u32, name="ei_all")      # top8 indices
    # selection data, plane index k = 2*g + q (group-major for batched gather)
    red_sb = persist.tile([P, OWN_TILES, 2 * NSEL], f32, name="red_sb")
    redi_sb = persist.tile([P, OWN_TILES, NSEL], i32, name="redi_sb")
    ot = [persist.tile([P, D], f32, name=f"ot{j}") for j in range(OWN_TILES)]

    xgT = [persist.tile([P, C], bf16, name=f"xgT{d}") for d in range(KD)]
    xga = [persist.tile([P, D], bf16, name=f"xga{s}") for s in range(ST)]

    y_dram = [dram.tile([CHK, D], bf16, name=f"y_dram{g}") for g in range(NG)]
    y_all = [dram.tile([N_CORES * CHK, D], bf16, addr_space="Shared",
                       name=f"y_all{g}") for g in range(NG)]

    # ------------- router: every core computes ALL logits locally -------------
    # (full xT fp32 is an input; no logits collective on the critical path)
    with tc.tile_pool(name="router_sb", bufs=1, named_scope="router") as rpool, \
         tc.tile_pool(name="router_ps", bufs=1, space="PSUM") as rps:
        l_all = rpool.tile([P, NE], f32, name="l_all")
        xTv = xTf.rearrange("(k p) t -> p k t", p=P)
        NCH = T // TOK_PER_CORE  # 8 chunks of 512 tokens
        QT = TOK_PER_CORE // P   # 4 token tiles per chunk
        for c in range(NCH):
            xsc = rpool.tile([P, KD, TOK_PER_CORE], f32, tag="xsc", bufs=3,
                             name="xsc")
            for k in range(KD):
                nc.sync.dma_start(
                    xsc[:, k, :],
                    xTv[:, k, c * TOK_PER_CORE:(c + 1) * TOK_PER_CORE])
            lT = rps.tile([E, TOK_PER_CORE], f32, tag="lT", bufs=2, name="lT")
            for k in range(KD):
                nc.tensor.matmul(lT[:], lhsT=wg_sb[:, k, :], rhs=xsc[:, k, :],
                                 start=(k == 0), stop=(k == KD - 1))
            lsb = rpool.tile([E, TOK_PER_CORE], f32, tag="lsb", bufs=2,
                             name="lsb")
            nc.vector.tensor_copy(lsb[:], lT[:])
            pt_l = rps.tile([P, QT, E], f32, tag="pt_l", bufs=2, name="pt_l")
            for q in range(QT):
                nc.tensor.transpose(
                    pt_l[:, q, :], lsb[:, q * P:(q + 1) * P], identf[:E, :E])
            nc.vector.tensor_copy(
                l_all[:, c * QT * E:(c + 1) * QT * E], pt_l[:])
        nc.vector.tensor_add(l_all[:], l_all[:], bg_sb[:])
        if dbg is not None:
            nc.sync.dma_start(dbg["dbg_l"][:], l_all[:])
        nc.scalar.activation(exp_all[:], l_all[:],
                             mybir.ActivationFunctionType.Exp)
        # top-2 indicator via masked second-max (5 full-width ops)
        exp3 = exp_all[:].rearrange("p (t e) -> p t e", e=E)
        mx = rpool.tile([P, NT], f32, name="mx")
        nc.vector.reduce_max(mx[:], exp3, axis=mybir.AxisListType.X)
        eqm = rpool.tile([P, NE], f32, name="eqm")
        nc.vector.tensor_tensor(
            out=eqm[:].rearrange("p (t e) -> p t e", e=E), in0=exp3,
            in1=mx[:, :, None].to_broadcast([P, NT, E]),
            op=mybir.AluOpType.is_ge)
        nm = rpool.tile([P, NE], f32, name="nm")
        nc.vector.scalar_tensor_tensor(
            out=nm[:], in0=eqm[:], scalar=-BIGP, in1=exp_all[:],
            op0=mybir.AluOpType.mult, op1=mybir.AluOpType.add)
        m2 = rpool.tile([P, NT], f32, name="m2")
        nc.vector.reduce_max(m2[:], nm[:].rearrange("p (t e) -> p t e", e=E),
                             axis=mybir.AxisListType.X)
        nc.vector.tensor_tensor(
            out=ind_all[:].rearrange("p (t e) -> p t e", e=E), in0=exp3,
            in1=m2[:, :, None].to_broadcast([P, NT, E]),
            op=mybir.AluOpType.is_ge)

    # expert weights: queued after the router's xT chunks so they don't
    # pace the router; done well before the FFN needs them
    w1_sb = [persist.tile([P, F], bf16, name=f"w1_sb{k}") for k in range(KD)]
    for k in range(KD):
        nc.sync.dma_start(w1_sb[k][:], w1d[k * P:(k + 1) * P, :])

    # ---------------- dispatch: cumsum positions + idx + gathers ----------------
    with tc.tile_pool(name="disp_sb", bufs=1, named_scope="dispatch") as dpool, \
         tc.tile_pool(name="disp_ps", bufs=1, space="PSUM") as dps:
        # PSUM is bank-granular (2KB/tile): pack the small intermediates
        # into two shared scratch banks addressed by column slices
        ps1 = dps.tile([P, 512], f32, name="ps1")
        ps2 = dps.tile([P, 512], f32, name="ps2")
        ptot_s = ps1[0:1, 0:NE]
        ppos_s = ps1[:, NE:2 * NE]
        t32_s = ps2[0:NT, 0:E]
        pofs_s = ps2[0:NT, E:2 * E]
        ofsT_s = ps2[0:E, 16:16 + NT]
        obc_s = ps2[:, 48:48 + NT]
        ppos2_s = ps2[:, 128:128 + NE]
        # per-(tile,expert) totals in one matmul
        nc.tensor.matmul(ptot_s, lhsT=ones_col[:], rhs=ind_all[:],
                         start=True, stop=True)
        tot_flat = dpool.tile([1, NE], f32, name="tot_flat")
        nc.vector.tensor_copy(tot_flat[:], ptot_s)
        # reshape [1, (t e)] -> [NT, E] with 8 strided transposes (no DMA)
        tf3 = tot_flat[:].rearrange("o (t e) -> o t e", e=E)
        for e in range(E):
            nc.tensor.transpose(t32_s[:, e:e + 1], tf3[:, :, e],
                                identf[:1, :1])
        tot32 = dpool.tile([NT, E], f32, name="tot32")
        nc.vector.tensor_copy(tot32[:], t32_s)
        # exclusive cumsum over tiles: strict-lower matmul
        nc.tensor.matmul(pofs_s, lhsT=l128_sb[:NT, :NT], rhs=tot32[:],
                         start=True, stop=True)
        ofs32 = dpool.tile([NT, E], f32, name="ofs32")
        nc.vector.tensor_copy(ofs32[:], pofs_s)
        # my expert's tile offsets broadcast over partitions:
        # transpose ofs32 -> [E, NT], then selE^T @ ofsT -> [P, NT]
        nc.tensor.transpose(ofsT_s, ofs32[:], identf[:NT, :NT])
        ofsT = dpool.tile([E, NT], f32, name="ofsT")
        nc.vector.tensor_copy(ofsT[:], ofsT_s)
        nc.tensor.matmul(obc_s, lhsT=selE_sb[:], rhs=ofsT[:],
                         start=True, stop=True)
        # tile-local exclusive cumsum (no tile offsets yet)
        nc.tensor.matmul(ppos_s, lhsT=l128_sb[:], rhs=ind_all[:],
                         start=True, stop=True)
        nc.vector.tensor_copy(pos_all[:], ppos_s)

        # fast masked positions for my expert:
        #   posm = ind_e ? (local_me + ofs_me) : BIGP
        t1 = dpool.tile([P, NE], f32, name="t1")
        nc.vector.tensor_mul(t1[:], ind_all[:], sel_sb[:])
        t2 = dpool.tile([P, NE], f32, name="t2")
        nc.vector.tensor_mul(t2[:], t1[:], pos_all[:])
        r1 = dpool.tile([P, NT], f32, name="r1")
        nc.vector.reduce_sum(r1[:], t2[:].rearrange(
            "p (t e) -> p t e", e=E), axis=mybir.AxisListType.X)
        ind_e = dpool.tile([P, NT], f32, name="ind_e")
        nc.vector.reduce_sum(ind_e[:], t1[:].rearrange(
            "p (t e) -> p t e", e=E), axis=mybir.AxisListType.X)
        pos_m = dpool.tile([P, NT], f32, name="pos_m")
        nc.vector.tensor_tensor(out=pos_m[:], in0=ind_e[:], in1=obc_s,
                                op=mybir.AluOpType.mult)
        nc.vector.tensor_add(pos_m[:], pos_m[:], r1[:])
        nc.vector.scalar_tensor_tensor(
            out=pos_m[:], in0=ind_e[:], scalar=-BIGP, in1=pos_m[:],
            op0=mybir.AluOpType.mult, op1=mybir.AluOpType.add)
        nc.vector.tensor_scalar_add(pos_m[:], pos_m[:], BIGP)

        # per group: one-hot matmul idx build -> finish -> gathers, so the
        # first FFN group's tokens arrive as early as possible
        idx_i = dpool.tile([P, ST], i32, name="idx_i")
        for g in range(NG):
            acc = dps.tile([2, CHK], f32, tag="accx", bufs=2, name="accx")
            for tt in range(NT):
                Pt = dpool.tile([P, CHK], bf16, tag="Pt", bufs=4, name="Pt")
                nc.vector.tensor_scalar(
                    Pt[:], iotaC[:, g * CHK:(g + 1) * CHK],
                    pos_m[:, tt:tt + 1], None,
                    op0=mybir.AluOpType.is_equal)
                nc.tensor.matmul(acc[:], lhsT=pv2[:, tt, :], rhs=Pt[:],
                                 start=(tt == 0), stop=(tt == NT - 1))
            idx2_sb = dpool.tile([2, CHK], f32, tag="idx2", bufs=2,
                                 name="idx2_sb")
            nc.vector.tensor_copy(idx2_sb[:], acc[:])
            pti_s = ps2[:, 48 + NT + 8 * g:48 + NT + 8 * g + 2 * TG]
            for t in range(TG):
                nc.tensor.transpose(pti_s[:, 2 * t:2 * t + 2],
                                    idx2_sb[:, t * P:(t + 1) * P],
                                    identf[:2, :2])
            pti_sb = dpool.tile([P, TG, 2], f32, tag="pti_sb", bufs=2,
                                name="pti_sb")
            nc.vector.tensor_copy(
                pti_sb[:].rearrange("p t o -> p (t o)"), pti_s)
            idx_f = dpool.tile([P, TG], f32, tag="idx_f", bufs=2,
                               name="idx_f")
            nc.vector.scalar_tensor_tensor(
                out=idx_f[:], in0=pti_sb[:, :, 1], scalar=float(P),
                in1=pti_sb[:, :, 0], op0=mybir.AluOpType.mult,
                op1=mybir.AluOpType.add)
            nc.vector.tensor_copy(idx_i[:, g * TG:(g + 1) * TG], idx_f[:])
            for t in range(TG):
                s = g * TG + t
                nc.gpsimd.indirect_dma_start(
                    out=xga[s][:], out_offset=None, in_=xbf[:],
                    in_offset=IndirectOffsetOnAxis(ap=idx_i[:, s:s + 1],
                                                   axis=0),
                )
        # finish full pos_all for the combine: add tile offsets via a
        # broadcast matmul (reshape DMA is off the critical path here)
        ofs_flat = dpool.tile([1, NE], f32, name="ofs_flat")
        nc.sync.dma_start(ofs_flat[:], ofs32[:])
        nc.tensor.matmul(ppos2_s, lhsT=ones_row[:], rhs=ofs_flat[:],
                         start=True, stop=True)
        nc.vector.tensor_tensor(out=pos_all[:], in0=pos_all[:],
                                in1=ppos2_s, op=mybir.AluOpType.add)
        if dbg is not None:
            nc.sync.dma_start(dbg["dbg_ind"][:], ind_all[:])
            nc.sync.dma_start(dbg["dbg_posm"][:], pos_m[:])
            nc.sync.dma_start(dbg["dbg_idx"][:], idx_i[:])
            nc.sync.dma_start(dbg["dbg_pos"][:], pos_all[:])

    # ---- transpose all gathered slot tiles into xgT on the TensorE ----
    with tc.tile_pool(name="gat_ps", bufs=4, space="PSUM") as gps:
        for s in range(ST):
            for d in range(KD):
                pt = gps.tile([P, P], bf16, tag="pt", bufs=4, name="pt")
                nc.tensor.transpose(pt[:], xga[s][:, d * P:(d + 1) * P],
                                    ident[:])
                nc.vector.tensor_copy(xgT[d][:, s * P:(s + 1) * P], pt[:])
        if dbg is not None:
            nc.sync.dma_start(dbg["dbg_xgT"][:], xgT[0][:])

    with tc.tile_pool(name="sel_sb", bufs=1, named_scope="select") as spool:
        # off the critical path: top-8 values/indices + softmax denom + the
        # combine selection stack (vector work that overlaps the scatter)
        for tt in range(NT):
            sl = slice(tt * E, (tt + 1) * E)
            nc.vector.max(out=m8_all[:, sl], in_=exp_all[:, sl])
            nc.vector.max_index(out=ei_all[:, sl], in_max=m8_all[:, sl],
                                in_values=exp_all[:, sl])
        s_all = spool.tile([P, NT], f32, name="s_all")
        nc.vector.reduce_sum(s_all[:], exp_all[:].rearrange(
            "p (t e) -> p t e", e=E), axis=mybir.AxisListType.X)
        nc.vector.reciprocal(r_all[:], s_all[:])

        # selection stack: NSEL offset planes + NSEL weight planes [P, NT],
        # plane k = 2*g + q (group-major so per-group gathers batch)
        e1f = spool.tile([P, NT], f32, name="e1f")
        e2f = spool.tile([P, NT], f32, name="e2f")
        ei3 = ei_all[:].rearrange("p (t e) -> p t e", e=E)
        nc.vector.tensor_copy(e1f[:], ei3[:, :, 0])
        nc.vector.tensor_copy(e2f[:], ei3[:, :, 1])
        ioz = spool.tile([P, NE], i32, name="ioz")
        nc.gpsimd.iota(ioz[:].rearrange("p (t e) -> p t e", e=E),
                       pattern=[[0, NT], [1, E]], base=0, channel_multiplier=0)
        iof = spool.tile([P, NE], f32, name="iof")
        nc.vector.tensor_copy(iof[:], ioz[:])
        m83 = m8_all[:].rearrange("p (t e) -> p t e", e=E)
        Ssel = spool.tile([P, 2 * NSEL, NT], f32, name="Ssel")
        for q, ef in ((0, e1f), (1, e2f)):
            oh = spool.tile([P, NE], f32, tag=f"oh{q}", name=f"oh{q}")
            nc.vector.tensor_tensor(
                out=oh[:].rearrange("p (t e) -> p t e", e=E),
                in0=iof[:].rearrange("p (t e) -> p t e", e=E),
                in1=ef[:, :, None].to_broadcast([P, NT, E]),
                op=mybir.AluOpType.is_equal)
            nc.vector.tensor_mul(oh[:], oh[:], pos_all[:])
            slot = spool.tile([P, NT], f32, tag=f"slot{q}", name=f"slot{q}")
            nc.vector.reduce_sum(slot[:], oh[:].rearrange(
                "p (t e) -> p t e", e=E), axis=mybir.AxisListType.X)
            gch = spool.tile([P, NT], f32, tag=f"gch{q}", name=f"gch{q}")
            nc.vector.tensor_scalar(gch[:], slot[:], float(CHK), None,
                                    op0=mybir.AluOpType.is_ge)
            for gg in range(2, NG):
                t2 = spool.tile([P, NT], f32, tag="t2", name="t2")
                nc.vector.tensor_scalar(t2[:], slot[:], float(CHK * gg), None,
                                        op0=mybir.AluOpType.is_ge)
                nc.vector.tensor_add(gch[:], gch[:], t2[:])
            base = spool.tile([P, NT], f32, tag=f"base{q}", name=f"base{q}")
            nc.vector.scalar_tensor_tensor(
                out=base[:], in0=ef[:], scalar=float(CHK), in1=slot[:],
                op0=mybir.AluOpType.mult, op1=mybir.AluOpType.add)
            gv = spool.tile([P, NT], f32, tag=f"gv{q}", name=f"gv{q}")
            nc.vector.tensor_tensor(out=gv[:], in0=m83[:, :, q], in1=r_all[:],
                                    op=mybir.AluOpType.mult)
            for gg in range(NG):
                k = 2 * gg + q
                eq = spool.tile([P, NT], f32, tag="eq", name="eq")
                nc.vector.tensor_scalar(eq[:], gch[:], float(gg), None,
                                        op0=mybir.AluOpType.is_equal)
                nc.vector.tensor_scalar_add(Ssel[:, k, :], base[:],
                                            float(-CHK * gg))
                nc.vector.tensor_mul(Ssel[:, k, :], Ssel[:, k, :], eq[:])
                nc.vector.tensor_mul(Ssel[:, NSEL + k, :], eq[:], gv[:])
        for j in range(OWN_TILES):
            own = ownm_sb[:, j * NT:(j + 1) * NT]
            tmpS = spool.tile([P, 2 * NSEL, NT], f32, tag="tmpS", bufs=2,
                              name="tmpS")
            nc.vector.tensor_tensor(
                out=tmpS[:], in0=Ssel[:],
                in1=own[:, None, :].to_broadcast([P, 2 * NSEL, NT]),
                op=mybir.AluOpType.mult)
            nc.vector.reduce_sum(red_sb[:, j, :], tmpS[:],
                                 axis=mybir.AxisListType.X)
            nc.vector.tensor_copy(redi_sb[:, j, :], red_sb[:, j, :NSEL])
        if dbg is not None:
            nc.sync.dma_start(
                dbg["dbg_red"][:].rearrange("p (j k) -> p j k", k=2 * NSEL),
                red_sb[:])


    # -------- FFN (bf16) + chunked y AllGather + overlapped combine --------
    with tc.tile_pool(name="ffn_sb", bufs=1, named_scope="ffn") as fpool, \
         tc.tile_pool(name="ffn_ps", bufs=1, space="PSUM") as fps, \
         tc.tile_pool(name="comb_sb", bufs=2, named_scope="combine") as cpool:
        for g in range(NG):
            t0 = g * TG
            py = [[fps.tile([P, 512], f32, tag=f"py_{t}_{n}",
                            name=f"py_{t}_{n}")
                   for n in range(2)] for t in range(TG)]
            for f in range(NF):
                ph = fps.tile([P, CHK], f32, tag="ph", bufs=2, name="ph")
                for k in range(KD):
                    nc.tensor.matmul(
                        ph[:], lhsT=w1_sb[k][:, f * P:(f + 1) * P],
                        rhs=xgT[k][:, t0 * P:t0 * P + CHK],
                        start=(k == 0), stop=(k == KD - 1))
                hbuf = fpool.tile([P, CHK], bf16, tag="hbuf", bufs=3,
                                  name="hbuf")
                nc.scalar.activation(hbuf[:], ph[:],
                                     mybir.ActivationFunctionType.Relu,
                                     bias=b1_sb[:, f:f + 1], scale=1.0)
                w2f = fpool.tile([P, D], bf16, tag="w2f", bufs=3, name="w2f")
                nc.sync.dma_start(w2f[:], w2d[f * P:(f + 1) * P, :])
                for t in range(TG):
                    for n in range(2):
                        nc.tensor.matmul(
                            py[t][n][:],
                            lhsT=hbuf[:, t * P:(t + 1) * P],
                            rhs=w2f[:, n * 512:(n + 1) * 512],
                            start=(f == 0), stop=False)
            # bias b2 via rank-1 matmul, then write out
            for t in range(TG):
                for n in range(2):
                    nc.tensor.matmul(
                        py[t][n][:], lhsT=ones_row[:],
                        rhs=b2_sb[:, n * 512:(n + 1) * 512],
                        start=False, stop=True)
                ysb = fpool.tile([P, D], bf16, tag="ysb", bufs=2, name="ysb")
                nc.vector.tensor_copy(ysb[:, :512], py[t][0][:])
                nc.vector.tensor_copy(ysb[:, 512:], py[t][1][:])
                nc.sync.dma_start(y_dram[g][t * P:(t + 1) * P, :], ysb[:])
            # ship this chunk while the next group computes
            nc.gpsimd.collective_compute(
                "AllGather", mybir.AluOpType.bypass,
                replica_groups=[list(range(N_CORES))],
                ins=[y_dram[g][:].opt()],
                outs=[y_all[g][:].opt()],
            )
            if dbg is not None:
                nc.sync.dma_start(dbg["dbg_y"][g][:], y_all[g][:])
            # combine this chunk: one batched gather of all (j, q) rows,
            # then gate-weighted accumulation while the next group runs
            yt = cpool.tile([P, OWN_TILES * 2, D], bf16, tag=f"yt{g}",
                            bufs=1, name=f"yt{g}")
            for j in range(OWN_TILES):
                for q in range(2):
                    nc.gpsimd.indirect_dma_start(
                        out=yt[:, 2 * j + q, :], out_offset=None,
                        in_=y_all[g][:],
                        in_offset=IndirectOffsetOnAxis(
                            ap=redi_sb[:, j, 2 * g + q:2 * g + q + 1],
                            axis=0))
            for j in range(OWN_TILES):
                for q in range(2):
                    k = 2 * g + q
                    w = red_sb[:, j, NSEL + k:NSEL + k + 1]
                    if g == 0 and q == 0:
                        nc.vector.tensor_scalar(ot[j][:], yt[:, 2 * j + q, :],
                                                w, None,
                                                op0=mybir.AluOpType.mult)
                    else:
                        nc.vector.scalar_tensor_tensor(
                            out=ot[j][:], in0=yt[:, 2 * j + q, :], scalar=w,
                            in1=ot[j][:], op0=mybir.AluOpType.mult,
                            op1=mybir.AluOpType.add)
        for j in range(OWN_TILES):
            nc.sync.dma_start(out[j * P:(j + 1) * P, :], ot[j][:])

    persist.release()
    dram.release()


def _host_prep(x, Wg, bg, W1, b1, W2, b2, C):
    xf = np.ascontiguousarray(x.reshape(T, D).astype(np.float32))
    xT = np.ascontiguousarray(xf.T)
    xbf = xf.astype(ml_dtypes.bfloat16)
    bgb = np.tile(bg.astype(np.float32), NT)[None, :].repeat(P, 0)
    bgb = np.ascontiguousarray(bgb)
    l128 = np.triu(np.ones((P, P), np.float32), 1)  # [t', t] = 1 if t' < t
    in_maps = []
    for c in range(N_CORES):
        sel = np.zeros(E, np.float32)
        sel[c] = 1.0
        sel256 = np.ascontiguousarray(np.tile(sel, NT)[None, :].repeat(P, 0))
        ownm = np.zeros((P, OWN_TILES, NT), np.float32)
        for j in range(OWN_TILES):
            ownm[:, j, OWN_TILES * c + j] = 1.0
        in_maps.append({
            "xTf": xT,
            "xbf": xbf,
            "w1d": np.ascontiguousarray(W1[c].astype(ml_dtypes.bfloat16)),
            "w2d": np.ascontiguousarray(W2[c].astype(ml_dtypes.bfloat16)),
            "wgd": np.ascontiguousarray(Wg.astype(np.float32)),
            "bgb": bgb,
            "b1pm": np.ascontiguousarray(
                b1[c].astype(np.float32).reshape(NF, P).T),
            "b2r": np.ascontiguousarray(b2[c].astype(np.float32)[None, :]),
            "sel256": sel256,
            "selEd": np.ascontiguousarray(
                sel[:, None].repeat(P, 1).astype(np.float32)),
            "l128d": l128,
            "ownmd": np.ascontiguousarray(ownm.reshape(P, OWN_TILES * NT)),
        })
    return in_maps


def _capacity(x, Wg, bg):
    xf = x.reshape(T, D).astype(np.float32)
    logits = xf @ Wg.astype(np.float32) + bg.astype(np.float32)
    part = np.partition(logits, E - 2, axis=-1)
    m2 = part[:, E - 2:E - 1]
    counts = (logits >= m2).sum(0)
    return int(np.ceil((counts.max() + 16) / CHK) * CHK)


LAST_RESULT = None


def kernel(x, Wg, bg, W1, b1, W2, b2):
    global LAST_RESULT
    from concourse.bass_utils import run_bass_kernel_spmd

    x = np.asarray(x)
    C = _capacity(x, np.asarray(Wg), np.asarray(bg))
    debug = bool(os.environ.get("BASS_DEBUG_OUT"))
    key = (C, debug)
    if key not in _cache:
        _cache[key] = build_module(C, debug_out=debug)
    nc = _cache[key]
    in_maps = _host_prep(x, np.asarray(Wg), np.asarray(bg), np.asarray(W1),
                         np.asarray(b1), np.asarray(W2), np.asarray(b2), C)
    trace = bool(os.environ.get("BASS_TRACE"))
    if trace:
        _setup_axon_profile_hook()
    res = run_bass_kernel_spmd(nc, in_maps, core_ids=list(range(N_CORES)),
                               trace=trace)
    LAST_RESULT = res
    out = np.empty((T, D), np.float32)
    for c in range(N_CORES):
        out[c * TOK_PER_CORE:(c + 1) * TOK_PER_CORE] = res.results[c]["out"]
    return out.reshape(B, S, D)


def _setup_axon_profile_hook():
    """Provide antenv.axon_hooks (missing in this image) so trace=True works."""
    import types
    try:
        import antenv
        if "antenv.axon_hooks" not in sys.modules:
            hooks = types.ModuleType("antenv.axon_hooks")
            hooks._hook = None
            hooks.set_axon_ntff_profile_hook = \
                lambda h: setattr(hooks, "_hook", h)
            hooks.get_axon_ntff_profile_hook = lambda: hooks._hook
            sys.modules["antenv.axon_hooks"] = hooks
            antenv.axon_hooks = hooks
            from trn_agent_boot.trn_boot import _ntff_profile_via_ctypes
            hooks.set_axon_ntff_profile_hook(
                _ntff_profile_via_ctypes("/opt/axon/libaxon_pjrt.so"))
    except Exception as e:  # profiling is best-effort
        print(f"profile hook setup failed: {e}", file=sys.stderr)
